# revision 1
# baseline (speedup 1.0000x reference)
"""MoE layer (top-2 routing, E=8 experts) on 8 Trainium2 NeuronCores.

Strategy (expert-parallel, per sharding hint):
 - Host computes the router (softmax over x@Wg+bg, top-2) and dispatches
   each (token, gate) pair to its expert's core: core e gets the tokens
   routed to expert e (gathered, transposed to [D, C], zero-padded to a
   common capacity C).
 - Core e runs a fused MLP kernel for expert e:
       hT = Gelu(W1^T-chunks @ xT + b1)   (PSUM -> SBUF via ACT, bias fused)
       y  = (h @ W2) * gate               (accumulated in PSUM across all
                                           24 F-chunks, gate fused on evict)
   Matmuls run in float32r (full-rate fp32) with fp32 PSUM accumulation.
 - Host scatter-adds the per-expert outputs back into [N, D] and adds the
   (separable) b2 term: sum_k gate_k * b2[e_k].
"""

import numpy as np

B, T, D = 4, 2048, 768
E, F, TOPK = 8, 4 * 768, 2
N = B * T
P = 128
TB = 384          # tokens per on-device block (3 x 128)
NCORES = 8

_nc_cache = {}


def _route(x_flat, Wg, bg):
    """Replicate reference routing: softmax gates, top-2 (ties -> lower idx)."""
    logits = x_flat.astype(np.float64) @ Wg.astype(np.float64) + bg.astype(np.float64)
    logits -= logits.max(axis=-1, keepdims=True)
    eg = np.exp(logits)
    gates = eg / eg.sum(axis=-1, keepdims=True)          # [N, E] f64
    top2 = np.argsort(-gates, axis=-1, kind="stable")[:, :TOPK]   # [N, 2]
    g2 = np.take_along_axis(gates, top2, axis=-1).astype(np.float32)
    return top2, g2


def _build_nc(C, TBo=None, w1q=4, hbufs=3, p1bufs=2, skip_wdma=False):
    import concourse.bacc as bacc
    import concourse.mybir as mybir
    import concourse.tile as tile

    f32 = mybir.dt.float32
    f32r = mybir.dt.float32r
    Gelu = mybir.ActivationFunctionType.Gelu

    KO1 = D // P          # 6 contraction chunks for x@W1
    KO2 = F // P          # 24 contraction chunks for h@W2
    # token blocks: 384s plus 256s so C only needs 128 granularity
    # (psum free dim must stay >= 256 for full-rate fp32r)
    assert C % 128 == 0 and C >= 256
    b384, rem = divmod(C, 384)
    if rem == 0:
        blocks = [384] * b384
    elif rem == 128:
        blocks = [384] * (b384 - 1) + [256, 256]
    else:
        blocks = [384] * b384 + [256]
    assert sum(blocks) == C
    DH = 2                # output D split (psum free <= 512 for f32)
    DHW = D // DH         # 384

    nc = bacc.Bacc("TRN2", target_bir_lowering=False)

    xT = nc.dram_tensor("xT", [D, C], f32r, kind="ExternalInput")
    w1 = nc.dram_tensor("w1", [D, F], f32r, kind="ExternalInput")
    b1 = nc.dram_tensor("b1", [F], f32, kind="ExternalInput")
    w2 = nc.dram_tensor("w2", [F, D], f32r, kind="ExternalInput")
    gates = nc.dram_tensor("gates", [C], f32, kind="ExternalInput")
    y = nc.dram_tensor("y", [C, D], f32, kind="ExternalOutput")

    with tile.TileContext(nc) as tc:
        with (
            tc.tile_pool(name="wpool", bufs=1) as wpool,
            tc.tile_pool(name="xpool", bufs=2) as xpool,
            tc.tile_pool(name="hpool", bufs=hbufs) as hpool,
            tc.tile_pool(name="ypool", bufs=2) as ypool,
            tc.tile_pool(name="psum1", bufs=p1bufs, space="PSUM") as psum1,
            tc.tile_pool(name="psumy", bufs=1, space="PSUM") as psumy,
        ):
            # Small constants first (needed by the first ACT/DVE evicts).
            b1_sb = wpool.tile([P, KO2], f32, tag="b1")
            nc.sync.dma_start(b1_sb[:], b1[:].rearrange("(fo p) -> p fo", p=P))
            gates_sb = wpool.tile([P, C // P], f32, tag="gates")
            nc.sync.dma_start(gates_sb[:], gates[:].rearrange("(mo p) -> p mo", p=P))
            # Resident weights, split per chunk and DMA'd in first-block
            # consumption order (w1 quarter q feeds fc in [q*6, q*6+6), then
            # w2[fc] for those fc) so the PE can start ~4 MB in instead of
            # waiting for the full 19 MB weight load.
            FQ = F // w1q
            w1_ap = w1[:, :].rearrange("(ko p) f -> ko p f", p=P)
            w2_ap = w2[:, :].rearrange("(ko p) d -> ko p d", p=P)
            w1_sb = [[None] * w1q for _ in range(KO1)]
            w2_sb = [None] * KO2
            for q in range(w1q):
                for kc in range(KO1):
                    t = wpool.tile([P, FQ], f32r, tag=f"w1_{kc}_{q}",
                                   name=f"w1sb_{kc}_{q}")
                    if not skip_wdma:
                        nc.sync.dma_start(t[:], w1_ap[kc][:, q * FQ:(q + 1) * FQ])
                    w1_sb[kc][q] = t
                for fc in range(q * (KO2 // w1q), (q + 1) * (KO2 // w1q)):
                    t = wpool.tile([P, D], f32r, tag=f"w2_{fc}", name=f"w2sb_{fc}")
                    if not skip_wdma:
                        nc.sync.dma_start(t[:], w2_ap[fc])
                    w2_sb[fc] = t

            xT_ap = xT[:, :].rearrange("(ko p) c -> p ko c", p=P)

            tok0 = 0
            for tb, TBl in enumerate(blocks):
                TS = TBl // P
                x_sb = xpool.tile([P, KO1, TBl], f32r, tag="x")
                nc.gpsimd.dma_start(x_sb[:], xT_ap[:, :, tok0:tok0 + TBl])

                ypsum = [
                    [
                        psumy.tile([P, DHW], f32, tag=f"y_{ts}_{dh}",
                                   name=f"ypsum_{ts}_{dh}")
                        for dh in range(DH)
                    ]
                    for ts in range(TS)
                ]
                # Software-pipeline: emit mm1 group LA chunks ahead of the
                # mm2 stream so the PE has fill work while the previous
                # block's y-psum banks drain (avoids in-order head-of-line
                # blocking at block boundaries).
                LA = 2
                h_q = {}
                for i in range(KO2 + LA):
                    if i < KO2:
                        fc = i
                        p1 = psum1.tile([P, TBl], f32, tag="p1")
                        for kc in range(KO1):
                            nc.tensor.matmul(
                                p1[:],
                                lhsT=w1_sb[kc][fc // (KO2 // w1q)][
                                    :, (fc % (KO2 // w1q)) * P:
                                    (fc % (KO2 // w1q) + 1) * P],
                                rhs=x_sb[:, kc, :],
                                start=(kc == 0),
                                stop=(kc == KO1 - 1),
                            )
                        h_sb = hpool.tile([P, TBl], f32r, tag="h")
                        nc.scalar.activation(h_sb[:], p1[:], Gelu,
                                             bias=b1_sb[:, fc:fc + 1])
                        h_q[fc] = h_sb
                    j = i - LA
                    if j >= 0:
                        h_j = h_q.pop(j)
                        for ts in range(TS):
                            for dh in range(DH):
                                nc.tensor.matmul(
                                    ypsum[ts][dh][:],
                                    lhsT=h_j[:, ts * P:(ts + 1) * P],
                                    rhs=w2_sb[j][:, dh * DHW:(dh + 1) * DHW],
                                    start=(j == 0),
                                    stop=(j == KO2 - 1),
                                )
                y_sb = ypool.tile([P, TS, D], f32, tag="y")
                mo0 = tok0 // P
                for ts in range(TS):
                    g_ap = gates_sb[:, mo0 + ts: mo0 + ts + 1]
                    for dh in range(DH):
                        nc.vector.tensor_scalar_mul(
                            y_sb[:, ts, dh * DHW:(dh + 1) * DHW],
                            ypsum[ts][dh][:],
                            g_ap,
                        )
                nc.gpsimd.dma_start(
                    y[tok0:tok0 + TBl, :].rearrange("(ts p) d -> p ts d", p=P),
                    y_sb[:],
                )
                tok0 += TBl
    nc.compile()
    return nc


def kernel(x, Wg, bg, W1, b1, W2, b2):
    from concourse.bass_utils import run_bass_kernel_spmd

    x = np.asarray(x, dtype=np.float32)
    Wg = np.asarray(Wg, dtype=np.float32)
    bg = np.asarray(bg, dtype=np.float32)
    W1 = np.asarray(W1, dtype=np.float32)
    b1 = np.asarray(b1, dtype=np.float32)
    W2 = np.asarray(W2, dtype=np.float32)
    b2 = np.asarray(b2, dtype=np.float32)

    x_flat = x.reshape(-1, D)
    top2, g2 = _route(x_flat, Wg, bg)

    # Dispatch: token lists per expert
    idx_e = []
    gate_e = []
    for e in range(E):
        sel = np.nonzero(top2 == e)
        idx_e.append(sel[0].astype(np.int64))                  # token ids
        gate_e.append(g2[sel[0], sel[1]].astype(np.float32))   # their gates
    counts = [len(i) for i in idx_e]
    C = max(max(counts), 129)
    C = ((C + P - 1) // P) * P

    if C not in _nc_cache:
        _nc_cache[C] = _build_nc(C)
    nc = _nc_cache[C]

    in_maps = []
    for e in range(E):
        n_e = counts[e]
        xTe = np.zeros((D, C), dtype=np.float32)
        if n_e:
            xTe[:, :n_e] = x_flat[idx_e[e]].T
        ge = np.zeros((C,), dtype=np.float32)
        ge[:n_e] = gate_e[e]
        in_maps.append({
            "xT": xTe,
            "w1": np.ascontiguousarray(W1[e]),
            "b1": np.ascontiguousarray(b1[e]),
            "w2": np.ascontiguousarray(W2[e]),
            "gates": ge,
        })

    res = run_bass_kernel_spmd(nc, in_maps, core_ids=list(range(NCORES)))

    out = np.zeros((N, D), dtype=np.float32)
    for e in range(E):
        n_e = counts[e]
        if n_e:
            out[idx_e[e]] += res.results[e]["y"][:n_e]
    # separable b2 term: sum_k gate_k * b2[e_k]
    if np.any(b2):
        out += g2[:, 0:1] * b2[top2[:, 0]] + g2[:, 1:2] * b2[top2[:, 1]]
    return out.reshape(B, T, D)



# revision 5
# speedup vs baseline: 1.2992x; 1.2992x over previous
"""MoE layer (top-2 routing, E=8 experts) on 8 Trainium2 NeuronCores.

Strategy (expert-parallel, per sharding hint):
 - Host computes the router (softmax over x@Wg+bg, top-2) and dispatches
   each (token, gate) pair to its expert's core: core e gets the tokens
   routed to expert e, zero-padded to a common capacity C.
 - Core e runs a fused MLP for expert e entirely in fp8 (e4m3) matmuls
   using the PE DoubleRow perf mode (2 contraction chunks per matmul at
   0.5 cycles/row).  To keep accuracy well inside the 2e-2 gate each
   GEMM is computed as a 3-term "split fp8" sum with hi/lo e4m3 operands:
       A@B ~= A_hi@B_hi + A_lo@B_hi + A_hi@B_lo       (rel err ~2e-3)
   where X_hi = e4m3(s*X), X_lo = e4m3(s*X - X_hi) at the same scale s.
   Per 128-contraction-chunk this costs 1.5 fp8 products = 0.75 of one
   fp32r matmul, on top of DoubleRow's 2x rate advantage.
 - Token-block pipeline per core: for each 256-token block
       mm1 (9 DoubleRow matmuls / 128-F-chunk) -> PSUM
       ACT: h = gelu(2^-13 * psum + b1)  (fp32), h_hi = e4m3(h)   [Scalar]
       DVE: h_lo = e4m3(h - h_hi)                                 [Vector]
       mm2 (3 DoubleRow matmuls / ko2-pair per (ts,dh)) -> PSUM
       DVE: y = psum * (gate * 2^-8)  -> SBUF -> DMA out
   mm1 of pair i+1 is emitted ahead of mm2 of pair i so the PE never
   stalls on the ACT/DVE h-split latency.
 - Host scatter-adds the per-expert outputs back into [N, D] and adds
   the separable b2 term: sum_k gate_k * b2[e_k].
"""

import numpy as np
import ml_dtypes

B, T, D = 4, 2048, 768
E, F, TOPK = 8, 4 * 768, 2
N = B * T
P = 128
NB = 256            # tokens per on-device block
NCORES = 8
KO1 = D // P        # 6 contraction chunks for x@W1
KO2 = F // P        # 24 contraction chunks for h@W2
JP2 = KO2 // 2      # 12 chunk-pairs for mm2
NQ1 = 4             # w1 DMA quarters (by F range)
NQ2 = 4             # w2 DMA quarters (by ko2 range)
FQ = F // NQ1       # 768
KQ2 = KO2 // NQ2    # 6
DH = 2
DHW = D // DH       # 384
SX = 32.0           # x quant scale (2^5)
SW = 256.0          # weight quant scale (2^8)
F8 = ml_dtypes.float8_e4m3

_nc_cache = {}


def _route(x_flat, Wg, bg):
    """Replicate reference routing: softmax gates, top-2 (ties -> lower idx)."""
    logits = x_flat.astype(np.float64) @ Wg.astype(np.float64) + bg.astype(np.float64)
    logits -= logits.max(axis=-1, keepdims=True)
    eg = np.exp(logits)
    gates = eg / eg.sum(axis=-1, keepdims=True)          # [N, E] f64
    top2 = np.argsort(-gates, axis=-1, kind="stable")[:, :TOPK]   # [N, 2]
    g2 = np.take_along_axis(gates, top2, axis=-1).astype(np.float32)
    return top2, g2


def _q8(a):
    return np.clip(a, -240.0, 240.0).astype(F8)


def _split8(a, s):
    """hi/lo e4m3 split of a at scale s (both at the same scale)."""
    hi = _q8(a * np.float32(s))
    lo = _q8(a * np.float32(s) - hi.astype(np.float32))
    return hi, lo


def _blocks_of(C):
    assert C % P == 0
    nfull, rem = divmod(C, NB)
    return [NB] * nfull + ([rem] if rem else [])


def _build_nc(C, act="Gelu"):
    import concourse.bacc as bacc
    import concourse.mybir as mybir
    import concourse.tile as tile

    f32 = mybir.dt.float32
    f8 = mybir.dt.float8e4
    Gelu = getattr(mybir.ActivationFunctionType, act)
    Copy = mybir.ActivationFunctionType.Copy
    DR = mybir.MatmulPerfMode.DoubleRow
    S1 = 1.0 / (SX * SW)

    blocks = _blocks_of(C)

    nc = bacc.Bacc("TRN2", target_bir_lowering=False)

    # DRAM inputs are pre-laid-out on the host as SBUF images.
    xq = nc.dram_tensor("xq", [P, KO1 * 2 * C], f8, kind="ExternalInput")
    w1q = nc.dram_tensor("w1q", [P, KO1 * 2 * F], f8, kind="ExternalInput")
    w2q = nc.dram_tensor("w2q", [P, KO2 * 2 * D], f8, kind="ExternalInput")
    b1 = nc.dram_tensor("b1", [F], f32, kind="ExternalInput")
    gates = nc.dram_tensor("gates", [C], f32, kind="ExternalInput")
    y = nc.dram_tensor("y", [P, (C // P) * D], f32, kind="ExternalOutput")

    w1_view = w1q[:, :].rearrange("p (ko hl f) -> p ko hl f", ko=KO1, hl=2)
    w2_view = w2q[:, :].rearrange("p (ko hl d) -> p ko hl d", ko=KO2, hl=2)

    with tile.TileContext(nc) as tc:
        with (
            tc.tile_pool(name="wpool", bufs=1) as wpool,
            tc.tile_pool(name="xpool", bufs=2) as xpool,
            tc.tile_pool(name="hpool", bufs=4) as hpool,
            tc.tile_pool(name="h32pool", bufs=3) as h32pool,
            tc.tile_pool(name="ypool", bufs=2) as ypool,
            tc.tile_pool(name="psum1", bufs=4, space="PSUM") as psum1,
            tc.tile_pool(name="psumy", bufs=1, space="PSUM") as psumy,
        ):
            # Small constants first (needed by the first ACT/DVE evicts).
            b1_sb = wpool.tile([P, KO2], f32, tag="b1")
            nc.sync.dma_start(b1_sb[:], b1[:].rearrange("(fo p) -> p fo", p=P))
            gates_sb = wpool.tile([P, C // P], f32, tag="gates")
            nc.sync.dma_start(gates_sb[:], gates[:].rearrange("(mo p) -> p mo", p=P))
            # Quantized weights, DMA'd in consumption order (w1 quarter q
            # feeds F-chunks [q*6, q*6+6); w2 quarter q feeds ko2 chunks
            # [q*6, q*6+6)) so the PE can start almost immediately.
            w1_sb = []
            w2_sb = []
            for q in range(NQ1):
                t1 = wpool.tile([P, KO1, 2, FQ], f8, tag=f"w1_{q}",
                                name=f"w1sb_{q}")
                nc.sync.dma_start(t1[:], w1_view[:, :, :, q * FQ:(q + 1) * FQ])
                w1_sb.append(t1)
                t2 = wpool.tile([P, KQ2, 2, D], f8, tag=f"w2_{q}",
                                name=f"w2sb_{q}")
                nc.sync.dma_start(t2[:], w2_view[:, q * KQ2:(q + 1) * KQ2])
                w2_sb.append(t2)

            tok0 = 0
            for tb, NBl in enumerate(blocks):
                TS = NBl // P
                mo0 = tok0 // P
                x_sb = xpool.tile([P, KO1, 2, NBl], f8, tag="x")
                off = KO1 * 2 * tok0
                nc.gpsimd.dma_start(
                    x_sb[:],
                    xq[:, off:off + KO1 * 2 * NBl].rearrange(
                        "p (ko hl n) -> p ko hl n", ko=KO1, hl=2),
                )

                ypsum = [
                    [
                        psumy.tile([P, DHW], f32, tag=f"y_{ts}_{dh}",
                                   name=f"ypsum_{ts}_{dh}")
                        for dh in range(DH)
                    ]
                    for ts in range(TS)
                ]
                # Pair-level software pipeline: mm1 for chunk-pair i runs one
                # step ahead of mm2 for pair i-1 so the PE never waits on the
                # ACT/DVE h-split.
                h_pairs = {}
                for i in range(JP2 + 1):
                    if i < JP2:
                        hp = hpool.tile([P, 2, 2, NBl], f8, tag="h")
                        for d in (0, 1):
                            fo = 2 * i + d
                            q, fr = divmod(fo, KQ2)
                            f0 = fr * P
                            w1t = w1_sb[q]
                            p1 = psum1.tile([P, NBl], f32, tag="p1")
                            nmm = KO1 // 2 + KO1
                            m = 0
                            # hi-hi terms: 2 K-chunks per DoubleRow matmul
                            for k in range(KO1 // 2):
                                nc.tensor.matmul(
                                    p1[:],
                                    lhsT=w1t[:, 2 * k:2 * k + 2, 0, f0:f0 + P],
                                    rhs=x_sb[:, 2 * k:2 * k + 2, 1, :],
                                    start=(m == 0), stop=(m == nmm - 1),
                                    perf_mode=DR,
                                )
                                m += 1
                            # cross terms: W1h.T@x_lo + W1l.T@x_hi per K-chunk
                            for k in range(KO1):
                                nc.tensor.matmul(
                                    p1[:],
                                    lhsT=w1t[:, k, 0:2, f0:f0 + P],
                                    rhs=x_sb[:, k, 0:2, :],
                                    start=(m == 0), stop=(m == nmm - 1),
                                    perf_mode=DR,
                                )
                                m += 1
                            h32 = h32pool.tile([P, NBl], f32, tag="h32")
                            nc.scalar.activation(h32[:], p1[:], Gelu,
                                                 bias=b1_sb[:, fo:fo + 1],
                                                 scale=S1)
                            nc.scalar.activation(hp[:, d, 1, :], h32[:], Copy)
                            nc.vector.tensor_sub(hp[:, d, 0, :], h32[:],
                                                 hp[:, d, 1, :])
                        h_pairs[i] = hp
                    j = i - 1
                    if j >= 0:
                        hp = h_pairs.pop(j)
                        qq, jr = divmod(2 * j, KQ2)
                        w2t = w2_sb[qq]
                        for ts in range(TS):
                            t0 = ts * P
                            for dh in range(DH):
                                d0 = dh * DHW
                                yp = ypsum[ts][dh]
                                # hi-hi: pair of consecutive ko2 chunks
                                nc.tensor.matmul(
                                    yp[:],
                                    lhsT=hp[:, 0:2, 1, t0:t0 + P],
                                    rhs=w2t[:, jr:jr + 2, 0, d0:d0 + DHW],
                                    start=(j == 0), stop=False,
                                    perf_mode=DR,
                                )
                                # cross: h_lo.T@W2h + h_hi.T@W2l per chunk
                                for d in (0, 1):
                                    nc.tensor.matmul(
                                        yp[:],
                                        lhsT=hp[:, d, 0:2, t0:t0 + P],
                                        rhs=w2t[:, jr + d, 0:2, d0:d0 + DHW],
                                        start=False,
                                        stop=(j == JP2 - 1 and d == 1),
                                        perf_mode=DR,
                                    )
                y_sb = ypool.tile([P, TS, D], f32, tag="y")
                for ts in range(TS):
                    g_ap = gates_sb[:, mo0 + ts: mo0 + ts + 1]
                    for dh in range(DH):
                        nc.vector.tensor_scalar_mul(
                            y_sb[:, ts, dh * DHW:(dh + 1) * DHW],
                            ypsum[ts][dh][:],
                            g_ap,
                        )
                nc.gpsimd.dma_start(
                    y[:, mo0 * D:(mo0 + TS) * D].rearrange(
                        "p (ts d) -> p ts d", ts=TS),
                    y_sb[:],
                )
                tok0 += NBl
    nc.compile()
    return nc


def kernel(x, Wg, bg, W1, b1, W2, b2):
    from concourse.bass_utils import run_bass_kernel_spmd

    x = np.asarray(x, dtype=np.float32)
    Wg = np.asarray(Wg, dtype=np.float32)
    bg = np.asarray(bg, dtype=np.float32)
    W1 = np.asarray(W1, dtype=np.float32)
    b1 = np.asarray(b1, dtype=np.float32)
    W2 = np.asarray(W2, dtype=np.float32)
    b2 = np.asarray(b2, dtype=np.float32)

    x_flat = x.reshape(-1, D)
    top2, g2 = _route(x_flat, Wg, bg)

    # Dispatch: token lists per expert
    idx_e = []
    gate_e = []
    for e in range(E):
        sel = np.nonzero(top2 == e)
        idx_e.append(sel[0].astype(np.int64))                  # token ids
        gate_e.append(g2[sel[0], sel[1]].astype(np.float32))   # their gates
    counts = [len(i) for i in idx_e]
    C = max(max(counts), 2 * P)
    C = ((C + P - 1) // P) * P

    if C not in _nc_cache:
        _nc_cache[C] = _build_nc(C)
    nc = _nc_cache[C]

    # Global hi/lo fp8 split of the tokens (shared across experts).
    Xh, Xl = _split8(x_flat, SX)      # [N, D] fp8 each
    blocks = _blocks_of(C)
    bl_off = np.cumsum([0] + blocks)[:-1]

    in_maps = []
    for e in range(E):
        n_e = counts[e]
        # x image: [P, ko, hl, C], hl = (lo, hi), then block-major flattened
        # to match the kernel's per-block contiguous DMA slices.
        xe = np.zeros((P, KO1, 2, C), dtype=F8)
        if n_e:
            xe[:, :, 0, :n_e] = (
                Xl[idx_e[e]].T.reshape(KO1, P, n_e).transpose(1, 0, 2))
            xe[:, :, 1, :n_e] = (
                Xh[idx_e[e]].T.reshape(KO1, P, n_e).transpose(1, 0, 2))
        xe = np.concatenate(
            [xe[:, :, :, o:o + nb].reshape(P, -1)
             for o, nb in zip(bl_off, blocks)], axis=1)
        # w1 image: [P, ko, hl, F], hl = (hi, lo)
        w1h, w1l = _split8(W1[e], SW)      # [D, F]
        w1e = np.empty((P, KO1, 2, F), dtype=F8)
        w1e[:, :, 0, :] = w1h.reshape(KO1, P, F).transpose(1, 0, 2)
        w1e[:, :, 1, :] = w1l.reshape(KO1, P, F).transpose(1, 0, 2)
        # w2 image: [P, ko2, hl, D], hl = (hi, lo)
        w2h, w2l = _split8(W2[e], SW)      # [F, D]
        w2e = np.empty((P, KO2, 2, D), dtype=F8)
        w2e[:, :, 0, :] = w2h.reshape(KO2, P, D).transpose(1, 0, 2)
        w2e[:, :, 1, :] = w2l.reshape(KO2, P, D).transpose(1, 0, 2)
        ge = np.zeros((C,), dtype=np.float32)
        ge[:n_e] = gate_e[e] * np.float32(1.0 / SW)
        in_maps.append({
            "xq": xe,
            "w1q": w1e.reshape(P, -1),
            "w2q": w2e.reshape(P, -1),
            "b1": np.ascontiguousarray(b1[e]),
            "gates": ge,
        })

    res = run_bass_kernel_spmd(nc, in_maps, core_ids=list(range(NCORES)))

    out = np.zeros((N, D), dtype=np.float32)
    for e in range(E):
        n_e = counts[e]
        if n_e:
            ye = res.results[e]["y"].reshape(P, C // P, D)
            ye = ye.transpose(1, 0, 2).reshape(C, D)
            out[idx_e[e]] += ye[:n_e]
    # separable b2 term: sum_k gate_k * b2[e_k]
    if np.any(b2):
        out += g2[:, 0:1] * b2[top2[:, 0]] + g2[:, 1:2] * b2[top2[:, 1]]
    return out.reshape(B, T, D)


# revision 53
# speedup vs baseline: 1.3332x; 1.0262x over previous
"""MoE layer (top-2 routing, E=8 experts) on 8 Trainium2 NeuronCores.

Strategy (expert-parallel, per sharding hint):
 - Host computes the router (softmax over x@Wg+bg, top-2) and dispatches
   each (token, gate) pair to its expert's core: core e gets the tokens
   routed to expert e, zero-padded to a common capacity C.
 - Core e runs a fused MLP for expert e entirely in fp8 (e4m3) matmuls
   using the PE DoubleRow perf mode (2 contraction chunks per matmul at
   0.5 cycles/row).  To keep accuracy well inside the 2e-2 gate each
   GEMM is computed as a 3-term "split fp8" sum with hi/lo e4m3 operands:
       A@B ~= A_hi@B_hi + A_lo@B_hi + A_hi@B_lo       (rel err ~2e-3)
   where X_hi = e4m3(s*X), X_lo = e4m3(s*X - X_hi) at the same scale s.
   Per 128-contraction-chunk this costs 1.5 fp8 products = 0.75 of one
   fp32r matmul, on top of DoubleRow's 2x rate advantage.
 - Token-block pipeline per core: for each 256-token block
       mm1 (9 DoubleRow matmuls / 128-F-chunk) -> PSUM
       ACT: h = gelu(2^-13 * psum + b1)  (fp32), h_hi = e4m3(h)   [Scalar]
       DVE: h_lo = e4m3(h - h_hi)                                 [Vector]
       mm2 (3 DoubleRow matmuls / ko2-pair per (ts,dh)) -> PSUM
       DVE: y = psum * (gate * 2^-8)  -> SBUF -> DMA out
   mm1 of pair i+1 is emitted ahead of mm2 of pair i so the PE never
   stalls on the ACT/DVE h-split latency.
 - Host scatter-adds the per-expert outputs back into [N, D] and adds
   the separable b2 term: sum_k gate_k * b2[e_k].
"""

import numpy as np
import ml_dtypes

B, T, D = 4, 2048, 768
E, F, TOPK = 8, 4 * 768, 2
N = B * T
P = 128
NB = 256            # tokens per on-device block
LA = 2              # mm1 chunk-pair lookahead over the mm2 stream
WARM = 0            # PE warmup dummy matmul count (0 = off)
NCORES = 8
KO1 = D // P        # 6 contraction chunks for x@W1
KO2 = F // P        # 24 contraction chunks for h@W2
JP2 = KO2 // 2      # 12 chunk-pairs for mm2
# Weight DMA piece bounds in 128-chunks (w1: F-chunks, w2: ko2-chunks).
# 6-chunk pieces balance DMA pipelining against per-DMA fixed overheads.
# w2 piece bounds must be even (mm2 chunk-pairs may not straddle pieces).
PIECES1 = [(0, 6), (6, 12), (12, 18), (18, 24)]
PIECES2 = [(0, 6), (6, 12), (12, 18), (18, 24)]
DH = 2
DHW = D // DH       # 384
SX = 32.0           # x quant scale (2^5)
SW = 256.0          # weight quant scale (2^8)
F8 = ml_dtypes.float8_e4m3

_nc_cache = {}


def _route(x_flat, Wg, bg):
    """Replicate reference routing: softmax gates, top-2 (ties -> lower idx)."""
    logits = x_flat.astype(np.float64) @ Wg.astype(np.float64) + bg.astype(np.float64)
    logits -= logits.max(axis=-1, keepdims=True)
    eg = np.exp(logits)
    gates = eg / eg.sum(axis=-1, keepdims=True)          # [N, E] f64
    top2 = np.argsort(-gates, axis=-1, kind="stable")[:, :TOPK]   # [N, 2]
    g2 = np.take_along_axis(gates, top2, axis=-1).astype(np.float32)
    return top2, g2


def _q8(a):
    return np.clip(a, -240.0, 240.0).astype(F8)


def _split8(a, s):
    """hi/lo e4m3 split of a at scale s (both at the same scale)."""
    hi = _q8(a * np.float32(s))
    lo = _q8(a * np.float32(s) - hi.astype(np.float32))
    return hi, lo


def _blocks_of(C):
    assert C % P == 0
    nfull, rem = divmod(C, NB)
    return [NB] * nfull + ([rem] if rem else [])


def _build_nc(C, act="Gelu", skip_wdma=False, skip_xydma=False, fuse=False):
    import concourse.bacc as bacc
    import concourse.mybir as mybir
    import concourse.tile as tile

    f32 = mybir.dt.float32
    f8 = mybir.dt.float8e4
    Gelu = getattr(mybir.ActivationFunctionType, act)
    Copy = mybir.ActivationFunctionType.Copy
    DR = mybir.MatmulPerfMode.DoubleRow
    S1 = 1.0 / (SX * SW)

    blocks = _blocks_of(C)

    nc = bacc.Bacc("TRN2", target_bir_lowering=False)

    # DRAM inputs are pre-laid-out on the host as SBUF images.
    xq = nc.dram_tensor("xq", [P, KO1 * 2 * C], f8, kind="ExternalInput")
    w1q = nc.dram_tensor("w1q", [P, KO1 * 2 * F], f8, kind="ExternalInput")
    w2q = nc.dram_tensor("w2q", [P, KO2 * 2 * D], f8, kind="ExternalInput")
    b1 = nc.dram_tensor("b1", [P, KO2], f32, kind="ExternalInput")
    gates = nc.dram_tensor("gates", [P, C // P], f32, kind="ExternalInput")
    y = nc.dram_tensor("y", [P, (C // P) * D], f32, kind="ExternalOutput")

    w2_view = w2q[:, :].rearrange("p (ko hl d) -> p ko hl d", ko=KO2, hl=2)

    with tile.TileContext(nc) as tc:
        with (
            tc.tile_pool(name="wpool", bufs=1) as wpool,
            tc.tile_pool(name="xpool", bufs=2) as xpool,
            tc.tile_pool(name="hpool", bufs=4) as hpool,
            tc.tile_pool(name="h32pool", bufs=3) as h32pool,
            tc.tile_pool(name="ypool", bufs=2) as ypool,
            tc.tile_pool(name="psum1", bufs=3, space="PSUM") as psum1,
            tc.tile_pool(name="psumy", bufs=1, space="PSUM") as psumy,
        ):
            b1_sb = wpool.tile([P, KO2], f32, tag="b1")
            gates_sb = wpool.tile([P, C // P], f32, tag="gates")
            # PE warmup: dummy DoubleRow matmuls on a zeroed tile keep the PE
            # "continuously busy" through the cost model's p-state ramp while
            # the first weight pieces are still in flight.
            if WARM:
                warm = wpool.tile([P, 2, P], f8, tag="warm")
                nc.vector.memset(warm[:], 0)
                wpsum = psum1.tile([P, NB], f32, tag="p1")
                for _ in range(WARM):
                    nc.tensor.matmul(wpsum[:, 0:P], lhsT=warm[:], rhs=warm[:],
                                     start=True, stop=True, perf_mode=DR)
            # Quantized weights, DMA'd in consumption order so the PE can
            # start almost immediately.
            # x block fetches go on the gpsimd (Pool) DGE, y stores on the
            # SP DGE: with both on one engine, block b+1's x fetch issues
            # in-order behind block b's y store and the PE stalls ~1.3us at
            # every block boundary.
            x_tiles = {}

            def fetch_x(tok0, NBl):
                t = xpool.tile([P, KO1, 2, NBl], f8, tag="x")
                off = KO1 * 2 * tok0
                if not skip_xydma:
                    nc.gpsimd.dma_start(
                        t[:],
                        xq[:, off:off + KO1 * 2 * NBl].rearrange(
                            "p (ko hl n) -> p ko hl n", ko=KO1, hl=2))
                return t

            # Block 0's tokens are needed before any weights: fetch first so
            # the transfer isn't queued behind the first weight pieces.
            x_tiles[0] = fetch_x(0, blocks[0])

            # Interleave w1/w2 pieces roughly in consumption order.
            order = []
            i1 = i2 = 0
            while i1 < len(PIECES1) or i2 < len(PIECES2):
                if i1 < len(PIECES1):
                    order.append((1, i1)); i1 += 1
                if i2 < len(PIECES2):
                    order.append((2, i2)); i2 += 1
            w1_off = np.cumsum(
                [0] + [KO1 * 2 * (b - a) * P for a, b in PIECES1])
            w1_by_chunk = {}
            w2_by_pair = {}
            for oi, (which, q) in enumerate(order):
                if oi == 1:
                    # First weight piece gets the head of the SP DGE queue;
                    # the small constants follow (needed slightly later).
                    nc.sync.dma_start(b1_sb[:], b1[:, :])
                    nc.sync.dma_start(gates_sb[:], gates[:, :])
                if which == 1:
                    a, b = PIECES1[q]
                    t1 = wpool.tile([P, KO1, 2, (b - a) * P], f8,
                                    tag=f"w1_{q}", name=f"w1sb_{q}")
                    if not skip_wdma:
                        # w1q is piece-major on the host: piece q's rows are
                        # contiguous, so descriptors stay >= 512B.
                        nc.sync.dma_start(
                            t1[:],
                            w1q[:, w1_off[q]:w1_off[q + 1]].rearrange(
                                "p (ko hl f) -> p ko hl f", ko=KO1, hl=2))
                    for fo in range(a, b):
                        w1_by_chunk[fo] = (t1, (fo - a) * P)
                else:
                    a, b = PIECES2[q]
                    t2 = wpool.tile([P, b - a, 2, D], f8,
                                    tag=f"w2_{q}", name=f"w2sb_{q}")
                    if not skip_wdma:
                        nc.sync.dma_start(t2[:], w2_view[:, a:b])
                    for j in range(a // 2, b // 2):
                        w2_by_pair[j] = (t2, 2 * j - a)

            tok0 = 0
            for tb, NBl in enumerate(blocks):
                TS = NBl // P
                mo0 = tok0 // P
                x_sb = x_tiles.pop(tb) if tb in x_tiles else fetch_x(tok0, NBl)

                ypsum = [
                    [
                        psumy.tile([P, DHW], f32, tag=f"y_{ts}_{dh}",
                                   name=f"ypsum_{ts}_{dh}")
                        for dh in range(DH)
                    ]
                    for ts in range(TS)
                ]
                # Pair-level software pipeline: mm1 for chunk-pair i runs LA
                # steps ahead of the mm2 stream so the PE waits neither on
                # the ACT/DVE h-split nor on the previous block's psum drain.
                h_pairs = {}
                for i in range(JP2 + LA):
                    if i < JP2:
                        hp = hpool.tile([P, 2, 2, NBl], f8, tag="h")
                        if fuse:
                            # b1 == 0: both F-chunks of the pair share one
                            # psum bank and one ACT/DVE pass (bias is a
                            # scalar, so this needs a uniform b1).
                            p1 = psum1.tile([P, 2, NBl], f32, tag="p1")
                            nmm = 2 * (KO1 // 2 + KO1)
                            m = 0
                            for d in (0, 1):
                                fo = 2 * i + d
                                w1t, f0 = w1_by_chunk[fo]
                                for k in range(KO1 // 2):
                                    nc.tensor.matmul(
                                        p1[:, d, :],
                                        lhsT=w1t[:, 2 * k:2 * k + 2, 0,
                                                 f0:f0 + P],
                                        rhs=x_sb[:, 2 * k:2 * k + 2, 1, :],
                                        start=(m == 0), stop=(m == nmm - 1),
                                        perf_mode=DR,
                                    )
                                    m += 1
                                for k in range(KO1):
                                    nc.tensor.matmul(
                                        p1[:, d, :],
                                        lhsT=w1t[:, k, 0:2, f0:f0 + P],
                                        rhs=x_sb[:, k, 0:2, :],
                                        start=(m == 0), stop=(m == nmm - 1),
                                        perf_mode=DR,
                                    )
                                    m += 1
                            h32 = h32pool.tile([P, 2, NBl], f32, tag="h32")
                            nc.scalar.activation(h32[:], p1[:], Gelu,
                                                 scale=S1)
                            nc.scalar.activation(hp[:, 0:2, 1, :], h32[:],
                                                 Copy)
                            nc.vector.tensor_sub(hp[:, 0:2, 0, :], h32[:],
                                                 hp[:, 0:2, 1, :])
                        else:
                            for d in (0, 1):
                                fo = 2 * i + d
                                w1t, f0 = w1_by_chunk[fo]
                                p1 = psum1.tile([P, NBl], f32, tag="p1")
                                nmm = KO1 // 2 + KO1
                                m = 0
                                # hi-hi: 2 K-chunks per DoubleRow matmul
                                for k in range(KO1 // 2):
                                    nc.tensor.matmul(
                                        p1[:],
                                        lhsT=w1t[:, 2 * k:2 * k + 2, 0,
                                                 f0:f0 + P],
                                        rhs=x_sb[:, 2 * k:2 * k + 2, 1, :],
                                        start=(m == 0), stop=(m == nmm - 1),
                                        perf_mode=DR,
                                    )
                                    m += 1
                                # cross: W1h.T@x_lo + W1l.T@x_hi per K-chunk
                                for k in range(KO1):
                                    nc.tensor.matmul(
                                        p1[:],
                                        lhsT=w1t[:, k, 0:2, f0:f0 + P],
                                        rhs=x_sb[:, k, 0:2, :],
                                        start=(m == 0), stop=(m == nmm - 1),
                                        perf_mode=DR,
                                    )
                                    m += 1
                                h32 = h32pool.tile([P, NBl], f32, tag="h32")
                                nc.scalar.activation(h32[:], p1[:], Gelu,
                                                     bias=b1_sb[:, fo:fo + 1],
                                                     scale=S1)
                                nc.scalar.activation(hp[:, d, 1, :], h32[:],
                                                     Copy)
                                nc.vector.tensor_sub(hp[:, d, 0, :], h32[:],
                                                     hp[:, d, 1, :])
                        h_pairs[i] = hp
                    j = i - LA
                    if j >= 0:
                        hp = h_pairs.pop(j)
                        w2t, jr = w2_by_pair[j]
                        for ts in range(TS):
                            t0 = ts * P
                            for dh in range(DH):
                                d0 = dh * DHW
                                yp = ypsum[ts][dh]
                                # hi-hi: pair of consecutive ko2 chunks
                                nc.tensor.matmul(
                                    yp[:],
                                    lhsT=hp[:, 0:2, 1, t0:t0 + P],
                                    rhs=w2t[:, jr:jr + 2, 0, d0:d0 + DHW],
                                    start=(j == 0), stop=False,
                                    perf_mode=DR,
                                )
                                # cross: h_lo.T@W2h + h_hi.T@W2l per chunk
                                for d in (0, 1):
                                    nc.tensor.matmul(
                                        yp[:],
                                        lhsT=hp[:, d, 0:2, t0:t0 + P],
                                        rhs=w2t[:, jr + d, 0:2, d0:d0 + DHW],
                                        start=False,
                                        stop=(j == JP2 - 1 and d == 1),
                                        perf_mode=DR,
                                    )
                y_sb = ypool.tile([P, TS, D], f32, tag="y")
                last = tb == len(blocks) - 1
                for ts in range(TS):
                    g_ap = gates_sb[:, mo0 + ts: mo0 + ts + 1]
                    nc.vector.tensor_scalar_mul(
                        y_sb[:, ts, 0:DHW], ypsum[ts][0][:], g_ap)
                    if last:
                        # final drain: the second half on the (now idle)
                        # scalar engine so the kernel tail isn't DVE-serial
                        nc.scalar.mul(y_sb[:, ts, DHW:D], ypsum[ts][1][:],
                                      g_ap)
                    else:
                        nc.vector.tensor_scalar_mul(
                            y_sb[:, ts, DHW:D], ypsum[ts][1][:], g_ap)
                if not skip_xydma:
                    yv = y[:, mo0 * D:(mo0 + TS) * D].rearrange(
                        "p (ts d) -> p ts d", ts=TS)
                    if last:
                        # final drain: ship each half as soon as its gate
                        # multiply lands, on separate DGE queues
                        nc.sync.dma_start(yv[:, :, 0:DHW],
                                          y_sb[:, :, 0:DHW])
                        nc.gpsimd.dma_start(yv[:, :, DHW:D],
                                            y_sb[:, :, DHW:D])
                    else:
                        nc.sync.dma_start(yv, y_sb[:])
                tok0 += NBl
    nc.compile()
    return nc


def kernel(x, Wg, bg, W1, b1, W2, b2):
    from concourse.bass_utils import run_bass_kernel_spmd

    x = np.asarray(x, dtype=np.float32)
    Wg = np.asarray(Wg, dtype=np.float32)
    bg = np.asarray(bg, dtype=np.float32)
    W1 = np.asarray(W1, dtype=np.float32)
    b1 = np.asarray(b1, dtype=np.float32)
    W2 = np.asarray(W2, dtype=np.float32)
    b2 = np.asarray(b2, dtype=np.float32)

    x_flat = x.reshape(-1, D)
    top2, g2 = _route(x_flat, Wg, bg)

    # Dispatch: token lists per expert
    idx_e = []
    gate_e = []
    for e in range(E):
        sel = np.nonzero(top2 == e)
        idx_e.append(sel[0].astype(np.int64))                  # token ids
        gate_e.append(g2[sel[0], sel[1]].astype(np.float32))   # their gates
    counts = [len(i) for i in idx_e]
    C = max(max(counts), 2 * P)
    C = ((C + P - 1) // P) * P

    fuse = False      # fused mm1 pairs measured slower (coarser ACT overlap)
    key = (C, fuse)
    if key not in _nc_cache:
        _nc_cache[key] = _build_nc(C, fuse=fuse)
    nc = _nc_cache[key]

    # Global hi/lo fp8 split of the tokens (shared across experts).
    Xh, Xl = _split8(x_flat, SX)      # [N, D] fp8 each
    blocks = _blocks_of(C)
    bl_off = np.cumsum([0] + blocks)[:-1]

    in_maps = []
    for e in range(E):
        n_e = counts[e]
        # x image: [P, ko, hl, C], hl = (lo, hi), then block-major flattened
        # to match the kernel's per-block contiguous DMA slices.
        xe = np.zeros((P, KO1, 2, C), dtype=F8)
        if n_e:
            xe[:, :, 0, :n_e] = (
                Xl[idx_e[e]].T.reshape(KO1, P, n_e).transpose(1, 0, 2))
            xe[:, :, 1, :n_e] = (
                Xh[idx_e[e]].T.reshape(KO1, P, n_e).transpose(1, 0, 2))
        xe = np.concatenate(
            [xe[:, :, :, o:o + nb].reshape(P, -1)
             for o, nb in zip(bl_off, blocks)], axis=1)
        # w1 image: [P, ko, hl, F], hl = (hi, lo), then piece-major
        # flattened to match the kernel's per-piece contiguous DMA slices.
        w1h, w1l = _split8(W1[e], SW)      # [D, F]
        w1e = np.empty((P, KO1, 2, F), dtype=F8)
        w1e[:, :, 0, :] = w1h.reshape(KO1, P, F).transpose(1, 0, 2)
        w1e[:, :, 1, :] = w1l.reshape(KO1, P, F).transpose(1, 0, 2)
        w1e = np.concatenate(
            [w1e[:, :, :, a * P:b * P].reshape(P, -1)
             for a, b in PIECES1], axis=1)
        # w2 image: [P, ko2, hl, D], hl = (hi, lo)
        w2h, w2l = _split8(W2[e], SW)      # [F, D]
        w2e = np.empty((P, KO2, 2, D), dtype=F8)
        w2e[:, :, 0, :] = w2h.reshape(KO2, P, D).transpose(1, 0, 2)
        w2e[:, :, 1, :] = w2l.reshape(KO2, P, D).transpose(1, 0, 2)
        ge = np.zeros((C,), dtype=np.float32)
        ge[:n_e] = gate_e[e] * np.float32(1.0 / SW)
        in_maps.append({
            "xq": xe,
            "w1q": w1e,
            "w2q": w2e.reshape(P, -1),
            "b1": np.ascontiguousarray(b1[e].reshape(KO2, P).T),
            "gates": np.ascontiguousarray(ge.reshape(C // P, P).T),
        })

    res = run_bass_kernel_spmd(nc, in_maps, core_ids=list(range(NCORES)))

    out = np.zeros((N, D), dtype=np.float32)
    for e in range(E):
        n_e = counts[e]
        if n_e:
            ye = res.results[e]["y"].reshape(P, C // P, D)
            ye = ye.transpose(1, 0, 2).reshape(C, D)
            out[idx_e[e]] += ye[:n_e]
    # separable b2 term: sum_k gate_k * b2[e_k]
    if np.any(b2):
        out += g2[:, 0:1] * b2[top2[:, 0]] + g2[:, 1:2] * b2[top2[:, 1]]
    return out.reshape(B, T, D)


# revision 55
# speedup vs baseline: 1.3533x; 1.0151x over previous
"""MoE layer (top-2 routing, E=8 experts) on 8 Trainium2 NeuronCores.

Strategy (expert-parallel, per sharding hint):
 - Host computes the router (softmax over x@Wg+bg, top-2) and dispatches
   each (token, gate) pair to its expert's core: core e gets the tokens
   routed to expert e, zero-padded to a common capacity C.
 - Core e runs a fused MLP for expert e entirely in fp8 (e4m3) matmuls
   using the PE DoubleRow perf mode (2 contraction chunks per matmul at
   0.5 cycles/row).  To keep accuracy well inside the 2e-2 gate each
   GEMM is computed as a 3-term "split fp8" sum with hi/lo e4m3 operands:
       A@B ~= A_hi@B_hi + A_lo@B_hi + A_hi@B_lo       (rel err ~2e-3)
   where X_hi = e4m3(s*X), X_lo = e4m3(s*X - X_hi) at the same scale s.
   Per 128-contraction-chunk this costs 1.5 fp8 products = 0.75 of one
   fp32r matmul, on top of DoubleRow's 2x rate advantage.
 - Token-block pipeline per core: for each 256-token block
       mm1 (9 DoubleRow matmuls / 128-F-chunk) -> PSUM
       ACT: h = gelu(2^-13 * psum + b1)  (fp32), h_hi = e4m3(h)   [Scalar]
       DVE: h_lo = e4m3(h - h_hi)                                 [Vector]
       mm2 (3 DoubleRow matmuls / ko2-pair per (ts,dh)) -> PSUM
       DVE: y = psum * (gate * 2^-8)  -> SBUF -> DMA out
   mm1 of pair i+1 is emitted ahead of mm2 of pair i so the PE never
   stalls on the ACT/DVE h-split latency.
 - Host scatter-adds the per-expert outputs back into [N, D] and adds
   the separable b2 term: sum_k gate_k * b2[e_k].
"""

import numpy as np
import ml_dtypes

B, T, D = 4, 2048, 768
E, F, TOPK = 8, 4 * 768, 2
N = B * T
P = 128
NB = 256            # tokens per on-device block
LA = 2              # mm1 chunk-pair lookahead over the mm2 stream
WARM = 0            # PE warmup dummy matmul count (0 = off)
NCORES = 8
KO1 = D // P        # 6 contraction chunks for x@W1
KO2 = F // P        # 24 contraction chunks for h@W2
JP2 = KO2 // 2      # 12 chunk-pairs for mm2
# Weight DMA piece bounds in 128-chunks (w1: F-chunks, w2: ko2-chunks).
# 6-chunk pieces balance DMA pipelining against per-DMA fixed overheads.
# w2 piece bounds must be even (mm2 chunk-pairs may not straddle pieces).
PIECES1 = [(0, 6), (6, 12), (12, 18), (18, 24)]
PIECES2 = [(0, 6), (6, 12), (12, 18), (18, 24)]
DH = 2
DHW = D // DH       # 384
SX = 32.0           # x quant scale (2^5)
SW = 256.0          # weight quant scale (2^8)
F8 = ml_dtypes.float8_e4m3

_nc_cache = {}


def _route(x_flat, Wg, bg):
    """Replicate reference routing: softmax gates, top-2 (ties -> lower idx)."""
    logits = x_flat.astype(np.float64) @ Wg.astype(np.float64) + bg.astype(np.float64)
    logits -= logits.max(axis=-1, keepdims=True)
    eg = np.exp(logits)
    gates = eg / eg.sum(axis=-1, keepdims=True)          # [N, E] f64
    top2 = np.argsort(-gates, axis=-1, kind="stable")[:, :TOPK]   # [N, 2]
    g2 = np.take_along_axis(gates, top2, axis=-1).astype(np.float32)
    return top2, g2


def _q8(a):
    return np.clip(a, -240.0, 240.0).astype(F8)


def _split8(a, s):
    """hi/lo e4m3 split of a at scale s (both at the same scale)."""
    hi = _q8(a * np.float32(s))
    lo = _q8(a * np.float32(s) - hi.astype(np.float32))
    return hi, lo


def _blocks_of(C):
    assert C % P == 0
    nfull, rem = divmod(C, NB)
    return [NB] * nfull + ([rem] if rem else [])


def _build_nc(C, act="Gelu", skip_wdma=False, skip_xydma=False, fuse=False):
    import concourse.bacc as bacc
    import concourse.mybir as mybir
    import concourse.tile as tile

    f32 = mybir.dt.float32
    f8 = mybir.dt.float8e4
    Gelu = getattr(mybir.ActivationFunctionType, act)
    Copy = mybir.ActivationFunctionType.Copy
    DR = mybir.MatmulPerfMode.DoubleRow
    S1 = 1.0 / (SX * SW)

    blocks = _blocks_of(C)

    nc = bacc.Bacc("TRN2", target_bir_lowering=False)

    # DRAM inputs are pre-laid-out on the host as SBUF images.
    xq = nc.dram_tensor("xq", [P, KO1 * 2 * C], f8, kind="ExternalInput")
    w1q = nc.dram_tensor("w1q", [P, KO1 * 2 * F], f8, kind="ExternalInput")
    w2q = nc.dram_tensor("w2q", [P, KO2 * 2 * D], f8, kind="ExternalInput")
    b1 = nc.dram_tensor("b1", [P, KO2], f32, kind="ExternalInput")
    gates = nc.dram_tensor("gates", [P, C // P], f32, kind="ExternalInput")
    y = nc.dram_tensor("y", [P, (C // P) * D], f32, kind="ExternalOutput")

    w2_view = w2q[:, :].rearrange("p (ko hl d) -> p ko hl d", ko=KO2, hl=2)

    with tile.TileContext(nc) as tc:
        with (
            tc.tile_pool(name="wpool", bufs=1) as wpool,
            tc.tile_pool(name="xpool", bufs=2) as xpool,
            tc.tile_pool(name="hpool", bufs=6) as hpool,
            tc.tile_pool(name="h32pool", bufs=4) as h32pool,
            tc.tile_pool(name="ypool", bufs=2) as ypool,
            tc.tile_pool(name="psum1", bufs=4, space="PSUM") as psum1,
            tc.tile_pool(name="psumy", bufs=1, space="PSUM") as psumy,
        ):
            b1_sb = wpool.tile([P, KO2], f32, tag="b1")
            gates_sb = wpool.tile([P, C // P], f32, tag="gates")
            # PE warmup: dummy DoubleRow matmuls on a zeroed tile keep the PE
            # "continuously busy" through the cost model's p-state ramp while
            # the first weight pieces are still in flight.
            if WARM:
                warm = wpool.tile([P, 2, P], f8, tag="warm")
                nc.vector.memset(warm[:], 0)
                wpsum = psum1.tile([P, NB], f32, tag="p1")
                for _ in range(WARM):
                    nc.tensor.matmul(wpsum[:, 0:P], lhsT=warm[:], rhs=warm[:],
                                     start=True, stop=True, perf_mode=DR)
            # Quantized weights, DMA'd in consumption order so the PE can
            # start almost immediately.
            # x block fetches go on the gpsimd (Pool) DGE, y stores on the
            # SP DGE: with both on one engine, block b+1's x fetch issues
            # in-order behind block b's y store and the PE stalls ~1.3us at
            # every block boundary.
            x_tiles = {}

            def fetch_x(tok0, NBl):
                t = xpool.tile([P, KO1, 2, NBl], f8, tag="x")
                off = KO1 * 2 * tok0
                if not skip_xydma:
                    nc.gpsimd.dma_start(
                        t[:],
                        xq[:, off:off + KO1 * 2 * NBl].rearrange(
                            "p (ko hl n) -> p ko hl n", ko=KO1, hl=2))
                return t

            # Block 0's tokens are needed before any weights: fetch first so
            # the transfer isn't queued behind the first weight pieces.
            x_tiles[0] = fetch_x(0, blocks[0])

            # Interleave w1/w2 pieces roughly in consumption order.
            order = []
            i1 = i2 = 0
            while i1 < len(PIECES1) or i2 < len(PIECES2):
                if i1 < len(PIECES1):
                    order.append((1, i1)); i1 += 1
                if i2 < len(PIECES2):
                    order.append((2, i2)); i2 += 1
            w1_off = np.cumsum(
                [0] + [KO1 * 2 * (b - a) * P for a, b in PIECES1])
            w1_by_chunk = {}
            w2_by_pair = {}
            for oi, (which, q) in enumerate(order):
                if oi == 1:
                    # First weight piece gets the head of the SP DGE queue;
                    # the small constants follow (needed slightly later).
                    nc.sync.dma_start(b1_sb[:], b1[:, :])
                    nc.sync.dma_start(gates_sb[:], gates[:, :])
                if which == 1:
                    a, b = PIECES1[q]
                    t1 = wpool.tile([P, KO1, 2, (b - a) * P], f8,
                                    tag=f"w1_{q}", name=f"w1sb_{q}")
                    if not skip_wdma:
                        # w1q is piece-major on the host: piece q's rows are
                        # contiguous, so descriptors stay >= 512B.
                        nc.sync.dma_start(
                            t1[:],
                            w1q[:, w1_off[q]:w1_off[q + 1]].rearrange(
                                "p (ko hl f) -> p ko hl f", ko=KO1, hl=2))
                    for fo in range(a, b):
                        w1_by_chunk[fo] = (t1, (fo - a) * P)
                else:
                    a, b = PIECES2[q]
                    t2 = wpool.tile([P, b - a, 2, D], f8,
                                    tag=f"w2_{q}", name=f"w2sb_{q}")
                    if not skip_wdma:
                        nc.sync.dma_start(t2[:], w2_view[:, a:b])
                    for j in range(a // 2, b // 2):
                        w2_by_pair[j] = (t2, 2 * j - a)

            tok0 = 0
            for tb, NBl in enumerate(blocks):
                TS = NBl // P
                mo0 = tok0 // P
                x_sb = x_tiles.pop(tb) if tb in x_tiles else fetch_x(tok0, NBl)

                ypsum = [
                    [
                        psumy.tile([P, DHW], f32, tag=f"y_{ts}_{dh}",
                                   name=f"ypsum_{ts}_{dh}")
                        for dh in range(DH)
                    ]
                    for ts in range(TS)
                ]
                # Pair-level software pipeline: mm1 for chunk-pair i runs LA
                # steps ahead of the mm2 stream so the PE waits neither on
                # the ACT/DVE h-split nor on the previous block's psum drain.
                h_pairs = {}
                for i in range(JP2 + LA):
                    if i < JP2:
                        hp = hpool.tile([P, 2, 2, NBl], f8, tag="h")
                        if fuse:
                            # b1 == 0: both F-chunks of the pair share one
                            # psum bank and one ACT/DVE pass (bias is a
                            # scalar, so this needs a uniform b1).
                            p1 = psum1.tile([P, 2, NBl], f32, tag="p1")
                            nmm = 2 * (KO1 // 2 + KO1)
                            m = 0
                            for d in (0, 1):
                                fo = 2 * i + d
                                w1t, f0 = w1_by_chunk[fo]
                                for k in range(KO1 // 2):
                                    nc.tensor.matmul(
                                        p1[:, d, :],
                                        lhsT=w1t[:, 2 * k:2 * k + 2, 0,
                                                 f0:f0 + P],
                                        rhs=x_sb[:, 2 * k:2 * k + 2, 1, :],
                                        start=(m == 0), stop=(m == nmm - 1),
                                        perf_mode=DR,
                                    )
                                    m += 1
                                for k in range(KO1):
                                    nc.tensor.matmul(
                                        p1[:, d, :],
                                        lhsT=w1t[:, k, 0:2, f0:f0 + P],
                                        rhs=x_sb[:, k, 0:2, :],
                                        start=(m == 0), stop=(m == nmm - 1),
                                        perf_mode=DR,
                                    )
                                    m += 1
                            h32 = h32pool.tile([P, 2, NBl], f32, tag="h32")
                            nc.scalar.activation(h32[:], p1[:], Gelu,
                                                 scale=S1)
                            nc.scalar.activation(hp[:, 0:2, 1, :], h32[:],
                                                 Copy)
                            nc.vector.tensor_sub(hp[:, 0:2, 0, :], h32[:],
                                                 hp[:, 0:2, 1, :])
                        else:
                            for d in (0, 1):
                                fo = 2 * i + d
                                w1t, f0 = w1_by_chunk[fo]
                                p1 = psum1.tile([P, NBl], f32, tag="p1")
                                nmm = KO1 // 2 + KO1
                                m = 0
                                # hi-hi: 2 K-chunks per DoubleRow matmul
                                for k in range(KO1 // 2):
                                    nc.tensor.matmul(
                                        p1[:],
                                        lhsT=w1t[:, 2 * k:2 * k + 2, 0,
                                                 f0:f0 + P],
                                        rhs=x_sb[:, 2 * k:2 * k + 2, 1, :],
                                        start=(m == 0), stop=(m == nmm - 1),
                                        perf_mode=DR,
                                    )
                                    m += 1
                                # cross: W1h.T@x_lo + W1l.T@x_hi per K-chunk
                                for k in range(KO1):
                                    nc.tensor.matmul(
                                        p1[:],
                                        lhsT=w1t[:, k, 0:2, f0:f0 + P],
                                        rhs=x_sb[:, k, 0:2, :],
                                        start=(m == 0), stop=(m == nmm - 1),
                                        perf_mode=DR,
                                    )
                                    m += 1
                                h32 = h32pool.tile([P, NBl], f32, tag="h32")
                                nc.scalar.activation(h32[:], p1[:], Gelu,
                                                     bias=b1_sb[:, fo:fo + 1],
                                                     scale=S1)
                                nc.scalar.activation(hp[:, d, 1, :], h32[:],
                                                     Copy)
                                nc.vector.tensor_sub(hp[:, d, 0, :], h32[:],
                                                     hp[:, d, 1, :])
                        h_pairs[i] = hp
                    j = i - LA
                    if j >= 0:
                        hp = h_pairs.pop(j)
                        w2t, jr = w2_by_pair[j]
                        for ts in range(TS):
                            t0 = ts * P
                            for dh in range(DH):
                                d0 = dh * DHW
                                yp = ypsum[ts][dh]
                                # hi-hi: pair of consecutive ko2 chunks
                                nc.tensor.matmul(
                                    yp[:],
                                    lhsT=hp[:, 0:2, 1, t0:t0 + P],
                                    rhs=w2t[:, jr:jr + 2, 0, d0:d0 + DHW],
                                    start=(j == 0), stop=False,
                                    perf_mode=DR,
                                )
                                # cross: h_lo.T@W2h + h_hi.T@W2l per chunk
                                for d in (0, 1):
                                    nc.tensor.matmul(
                                        yp[:],
                                        lhsT=hp[:, d, 0:2, t0:t0 + P],
                                        rhs=w2t[:, jr + d, 0:2, d0:d0 + DHW],
                                        start=False,
                                        stop=(j == JP2 - 1 and d == 1),
                                        perf_mode=DR,
                                    )
                y_sb = ypool.tile([P, TS, D], f32, tag="y")
                last = tb == len(blocks) - 1
                for ts in range(TS):
                    g_ap = gates_sb[:, mo0 + ts: mo0 + ts + 1]
                    nc.vector.tensor_scalar_mul(
                        y_sb[:, ts, 0:DHW], ypsum[ts][0][:], g_ap)
                    if last:
                        # final drain: the second half on the (now idle)
                        # scalar engine so the kernel tail isn't DVE-serial
                        nc.scalar.mul(y_sb[:, ts, DHW:D], ypsum[ts][1][:],
                                      g_ap)
                    else:
                        nc.vector.tensor_scalar_mul(
                            y_sb[:, ts, DHW:D], ypsum[ts][1][:], g_ap)
                if not skip_xydma:
                    yv = y[:, mo0 * D:(mo0 + TS) * D].rearrange(
                        "p (ts d) -> p ts d", ts=TS)
                    if last:
                        # final drain: ship each half as soon as its gate
                        # multiply lands, on separate DGE queues
                        nc.sync.dma_start(yv[:, :, 0:DHW],
                                          y_sb[:, :, 0:DHW])
                        nc.gpsimd.dma_start(yv[:, :, DHW:D],
                                            y_sb[:, :, DHW:D])
                    else:
                        nc.sync.dma_start(yv, y_sb[:])
                tok0 += NBl
    nc.compile()
    return nc


def kernel(x, Wg, bg, W1, b1, W2, b2):
    from concourse.bass_utils import run_bass_kernel_spmd

    x = np.asarray(x, dtype=np.float32)
    Wg = np.asarray(Wg, dtype=np.float32)
    bg = np.asarray(bg, dtype=np.float32)
    W1 = np.asarray(W1, dtype=np.float32)
    b1 = np.asarray(b1, dtype=np.float32)
    W2 = np.asarray(W2, dtype=np.float32)
    b2 = np.asarray(b2, dtype=np.float32)

    x_flat = x.reshape(-1, D)
    top2, g2 = _route(x_flat, Wg, bg)

    # Dispatch: token lists per expert
    idx_e = []
    gate_e = []
    for e in range(E):
        sel = np.nonzero(top2 == e)
        idx_e.append(sel[0].astype(np.int64))                  # token ids
        gate_e.append(g2[sel[0], sel[1]].astype(np.float32))   # their gates
    counts = [len(i) for i in idx_e]
    C = max(max(counts), 2 * P)
    C = ((C + P - 1) // P) * P

    if C not in _nc_cache:
        # fuse=False: fused mm1 pairs measured slower (coarser ACT overlap)
        _nc_cache[C] = _build_nc(C, fuse=False)
    nc = _nc_cache[C]

    # Global hi/lo fp8 split of the tokens (shared across experts).
    Xh, Xl = _split8(x_flat, SX)      # [N, D] fp8 each
    blocks = _blocks_of(C)
    bl_off = np.cumsum([0] + blocks)[:-1]

    in_maps = []
    for e in range(E):
        n_e = counts[e]
        # x image: [P, ko, hl, C], hl = (lo, hi), then block-major flattened
        # to match the kernel's per-block contiguous DMA slices.
        xe = np.zeros((P, KO1, 2, C), dtype=F8)
        if n_e:
            xe[:, :, 0, :n_e] = (
                Xl[idx_e[e]].T.reshape(KO1, P, n_e).transpose(1, 0, 2))
            xe[:, :, 1, :n_e] = (
                Xh[idx_e[e]].T.reshape(KO1, P, n_e).transpose(1, 0, 2))
        xe = np.concatenate(
            [xe[:, :, :, o:o + nb].reshape(P, -1)
             for o, nb in zip(bl_off, blocks)], axis=1)
        # w1 image: [P, ko, hl, F], hl = (hi, lo), then piece-major
        # flattened to match the kernel's per-piece contiguous DMA slices.
        w1h, w1l = _split8(W1[e], SW)      # [D, F]
        w1e = np.empty((P, KO1, 2, F), dtype=F8)
        w1e[:, :, 0, :] = w1h.reshape(KO1, P, F).transpose(1, 0, 2)
        w1e[:, :, 1, :] = w1l.reshape(KO1, P, F).transpose(1, 0, 2)
        w1e = np.concatenate(
            [w1e[:, :, :, a * P:b * P].reshape(P, -1)
             for a, b in PIECES1], axis=1)
        # w2 image: [P, ko2, hl, D], hl = (hi, lo)
        w2h, w2l = _split8(W2[e], SW)      # [F, D]
        w2e = np.empty((P, KO2, 2, D), dtype=F8)
        w2e[:, :, 0, :] = w2h.reshape(KO2, P, D).transpose(1, 0, 2)
        w2e[:, :, 1, :] = w2l.reshape(KO2, P, D).transpose(1, 0, 2)
        ge = np.zeros((C,), dtype=np.float32)
        ge[:n_e] = gate_e[e] * np.float32(1.0 / SW)
        in_maps.append({
            "xq": xe,
            "w1q": w1e,
            "w2q": w2e.reshape(P, -1),
            "b1": np.ascontiguousarray(b1[e].reshape(KO2, P).T),
            "gates": np.ascontiguousarray(ge.reshape(C // P, P).T),
        })

    res = run_bass_kernel_spmd(nc, in_maps, core_ids=list(range(NCORES)))

    out = np.zeros((N, D), dtype=np.float32)
    for e in range(E):
        n_e = counts[e]
        if n_e:
            ye = res.results[e]["y"].reshape(P, C // P, D)
            ye = ye.transpose(1, 0, 2).reshape(C, D)
            out[idx_e[e]] += ye[:n_e]
    # separable b2 term: sum_k gate_k * b2[e_k]
    if np.any(b2):
        out += g2[:, 0:1] * b2[top2[:, 0]] + g2[:, 1:2] * b2[top2[:, 1]]
    return out.reshape(B, T, D)


# revision 63
# speedup vs baseline: 1.3551x; 1.0013x over previous
"""MoE layer (top-2 routing, E=8 experts) on 8 Trainium2 NeuronCores.

Strategy (expert-parallel, per sharding hint):
 - Host computes the router (softmax over x@Wg+bg, top-2) and dispatches
   each (token, gate) pair to its expert's core: core e gets the tokens
   routed to expert e, zero-padded to a common capacity C.
 - Core e runs a fused MLP for expert e entirely in fp8 (e4m3) matmuls
   using the PE DoubleRow perf mode (2 contraction chunks per matmul at
   0.5 cycles/row).  To keep accuracy well inside the 2e-2 gate each
   GEMM is computed as a 3-term "split fp8" sum with hi/lo e4m3 operands:
       A@B ~= A_hi@B_hi + A_lo@B_hi + A_hi@B_lo       (rel err ~2e-3)
   where X_hi = e4m3(s*X), X_lo = e4m3(s*X - X_hi) at the same scale s.
   Per 128-contraction-chunk this costs 1.5 fp8 products = 0.75 of one
   fp32r matmul, on top of DoubleRow's 2x rate advantage.
 - Token-block pipeline per core: for each 256-token block
       mm1 (9 DoubleRow matmuls / 128-F-chunk) -> PSUM
       ACT: h = gelu(2^-13 * psum + b1)  (fp32), h_hi = e4m3(h)   [Scalar]
       DVE: h_lo = e4m3(h - h_hi)                                 [Vector]
       mm2 (3 DoubleRow matmuls / ko2-pair per (ts,dh)) -> PSUM
       DVE: y = psum * (gate * 2^-8)  -> SBUF -> DMA out
   mm1 of pair i+1 is emitted ahead of mm2 of pair i so the PE never
   stalls on the ACT/DVE h-split latency.
 - Host scatter-adds the per-expert outputs back into [N, D] and adds
   the separable b2 term: sum_k gate_k * b2[e_k].
"""

import numpy as np
import ml_dtypes

B, T, D = 4, 2048, 768
E, F, TOPK = 8, 4 * 768, 2
N = B * T
P = 128
NB = 256            # tokens per on-device block
LA = 2              # mm1 chunk-pair lookahead over the mm2 stream
WARM = 0            # PE warmup dummy matmul count (0 = off)
NCORES = 8
KO1 = D // P        # 6 contraction chunks for x@W1
KO2 = F // P        # 24 contraction chunks for h@W2
JP2 = KO2 // 2      # 12 chunk-pairs for mm2
# Weight DMA piece bounds in 128-chunks (w1: F-chunks, w2: ko2-chunks).
# 6-chunk pieces balance DMA pipelining against per-DMA fixed overheads.
# w2 piece bounds must be even (mm2 chunk-pairs may not straddle pieces).
PIECES1 = [(0, 6), (6, 12), (12, 18), (18, 24)]
PIECES2 = [(0, 6), (6, 12), (12, 18), (18, 24)]
DH = 2
DHW = D // DH       # 384
SX = 32.0           # x quant scale (2^5)
SW = 256.0          # weight quant scale (2^8)
F8 = ml_dtypes.float8_e4m3

_nc_cache = {}


def _route(x_flat, Wg, bg):
    """Replicate reference routing: softmax gates, top-2 (ties -> lower idx)."""
    logits = x_flat.astype(np.float64) @ Wg.astype(np.float64) + bg.astype(np.float64)
    logits -= logits.max(axis=-1, keepdims=True)
    eg = np.exp(logits)
    gates = eg / eg.sum(axis=-1, keepdims=True)          # [N, E] f64
    top2 = np.argsort(-gates, axis=-1, kind="stable")[:, :TOPK]   # [N, 2]
    g2 = np.take_along_axis(gates, top2, axis=-1).astype(np.float32)
    return top2, g2


def _q8(a):
    return np.clip(a, -240.0, 240.0).astype(F8)


def _split8(a, s):
    """hi/lo e4m3 split of a at scale s (both at the same scale)."""
    hi = _q8(a * np.float32(s))
    lo = _q8(a * np.float32(s) - hi.astype(np.float32))
    return hi, lo


def _blocks_of(C):
    assert C % P == 0
    nfull, rem = divmod(C, NB)
    return [NB] * nfull + ([rem] if rem else [])


def _build_nc(C, act="Gelu", skip_wdma=False, skip_xydma=False, fuse=False):
    import concourse.bacc as bacc
    import concourse.mybir as mybir
    import concourse.tile as tile

    f32 = mybir.dt.float32
    f8 = mybir.dt.float8e4
    fy = mybir.dt.bfloat16
    Gelu = getattr(mybir.ActivationFunctionType, act)
    Copy = mybir.ActivationFunctionType.Copy
    DR = mybir.MatmulPerfMode.DoubleRow
    S1 = 1.0 / (SX * SW)

    blocks = _blocks_of(C)

    nc = bacc.Bacc("TRN2", target_bir_lowering=False)

    # DRAM inputs are pre-laid-out on the host as SBUF images.
    xq = nc.dram_tensor("xq", [P, KO1 * 2 * C], f8, kind="ExternalInput")
    w1q = nc.dram_tensor("w1q", [P, KO1 * 2 * F], f8, kind="ExternalInput")
    w2q = nc.dram_tensor("w2q", [P, KO2 * 2 * D], f8, kind="ExternalInput")
    b1 = nc.dram_tensor("b1", [P, KO2], f32, kind="ExternalInput")
    gates = nc.dram_tensor("gates", [P, C // P], f32, kind="ExternalInput")
    y = nc.dram_tensor("y", [P, (C // P) * D], fy, kind="ExternalOutput")

    w2_view = w2q[:, :].rearrange("p (ko hl d) -> p ko hl d", ko=KO2, hl=2)

    with tile.TileContext(nc) as tc:
        with (
            tc.tile_pool(name="wpool", bufs=1) as wpool,
            tc.tile_pool(name="xpool", bufs=2) as xpool,
            tc.tile_pool(name="hpool", bufs=JP2 + 2) as hpool,
            tc.tile_pool(name="h32pool", bufs=4) as h32pool,
            tc.tile_pool(name="ypool", bufs=2) as ypool,
            tc.tile_pool(name="psum1", bufs=4, space="PSUM") as psum1,
            tc.tile_pool(name="psumy", bufs=1, space="PSUM") as psumy,
        ):
            b1_sb = wpool.tile([P, KO2], f32, tag="b1")
            gates_sb = wpool.tile([P, C // P], f32, tag="gates")
            # PE warmup: dummy DoubleRow matmuls on a zeroed tile keep the PE
            # "continuously busy" through the cost model's p-state ramp while
            # the first weight pieces are still in flight.
            if WARM:
                warm = wpool.tile([P, 2, P], f8, tag="warm")
                nc.vector.memset(warm[:], 0)
                wpsum = psum1.tile([P, NB], f32, tag="p1")
                for _ in range(WARM):
                    nc.tensor.matmul(wpsum[:, 0:P], lhsT=warm[:], rhs=warm[:],
                                     start=True, stop=True, perf_mode=DR)
            # Quantized weights, DMA'd in consumption order so the PE can
            # start almost immediately.
            # x block fetches go on the gpsimd (Pool) DGE, y stores on the
            # SP DGE: with both on one engine, block b+1's x fetch issues
            # in-order behind block b's y store and the PE stalls ~1.3us at
            # every block boundary.
            x_tiles = {}

            def fetch_x(tok0, NBl):
                t = xpool.tile([P, KO1, 2, NBl], f8, tag="x")
                off = KO1 * 2 * tok0
                if not skip_xydma:
                    nc.gpsimd.dma_start(
                        t[:],
                        xq[:, off:off + KO1 * 2 * NBl].rearrange(
                            "p (ko hl n) -> p ko hl n", ko=KO1, hl=2))
                return t

            # Block 0's tokens are needed before any weights: fetch first so
            # the transfer isn't queued behind the first weight pieces.
            x_tiles[0] = fetch_x(0, blocks[0])

            # Interleave w1/w2 pieces roughly in consumption order.
            order = []
            i1 = i2 = 0
            while i1 < len(PIECES1) or i2 < len(PIECES2):
                if i1 < len(PIECES1):
                    order.append((1, i1)); i1 += 1
                if i2 < len(PIECES2):
                    order.append((2, i2)); i2 += 1
            w1_off = np.cumsum(
                [0] + [KO1 * 2 * (b - a) * P for a, b in PIECES1])
            w1_by_chunk = {}
            w2_by_pair = {}
            for oi, (which, q) in enumerate(order):
                if oi == 1:
                    # First weight piece gets the head of the SP DGE queue;
                    # the small constants follow (needed slightly later).
                    nc.sync.dma_start(b1_sb[:], b1[:, :])
                    nc.sync.dma_start(gates_sb[:], gates[:, :])
                if which == 1:
                    a, b = PIECES1[q]
                    t1 = wpool.tile([P, KO1, 2, (b - a) * P], f8,
                                    tag=f"w1_{q}", name=f"w1sb_{q}")
                    if not skip_wdma:
                        # w1q is piece-major on the host: piece q's rows are
                        # contiguous, so descriptors stay >= 512B.
                        nc.sync.dma_start(
                            t1[:],
                            w1q[:, w1_off[q]:w1_off[q + 1]].rearrange(
                                "p (ko hl f) -> p ko hl f", ko=KO1, hl=2))
                    for fo in range(a, b):
                        w1_by_chunk[fo] = (t1, (fo - a) * P)
                else:
                    a, b = PIECES2[q]
                    t2 = wpool.tile([P, b - a, 2, D], f8,
                                    tag=f"w2_{q}", name=f"w2sb_{q}")
                    if not skip_wdma:
                        nc.sync.dma_start(t2[:], w2_view[:, a:b])
                    for j in range(a // 2, b // 2):
                        w2_by_pair[j] = (t2, 2 * j - a)

            tok0 = 0
            for tb, NBl in enumerate(blocks):
                TS = NBl // P
                mo0 = tok0 // P
                last = tb == len(blocks) - 1
                x_sb = x_tiles.pop(tb) if tb in x_tiles else fetch_x(tok0, NBl)

                ypsum = [
                    [
                        psumy.tile([P, DHW], f32, tag=f"y_{ts}_{dh}",
                                   name=f"ypsum_{ts}_{dh}")
                        for dh in range(DH)
                    ]
                    for ts in range(TS)
                ]
                # Pair-level software pipeline: mm1 for chunk-pair i runs LA
                # steps ahead of the mm2 stream so the PE waits neither on
                # the ACT/DVE h-split nor on the previous block's psum drain.
                h_pairs = {}
                for i in range(JP2 + LA):
                    if i < JP2:
                        hp = hpool.tile([P, 2, 2, NBl], f8, tag="h")
                        if fuse:
                            # b1 == 0: both F-chunks of the pair share one
                            # psum bank and one ACT/DVE pass (bias is a
                            # scalar, so this needs a uniform b1).
                            p1 = psum1.tile([P, 2, NBl], f32, tag="p1")
                            nmm = 2 * (KO1 // 2 + KO1)
                            m = 0
                            for d in (0, 1):
                                fo = 2 * i + d
                                w1t, f0 = w1_by_chunk[fo]
                                for k in range(KO1 // 2):
                                    nc.tensor.matmul(
                                        p1[:, d, :],
                                        lhsT=w1t[:, 2 * k:2 * k + 2, 0,
                                                 f0:f0 + P],
                                        rhs=x_sb[:, 2 * k:2 * k + 2, 1, :],
                                        start=(m == 0), stop=(m == nmm - 1),
                                        perf_mode=DR,
                                    )
                                    m += 1
                                for k in range(KO1):
                                    nc.tensor.matmul(
                                        p1[:, d, :],
                                        lhsT=w1t[:, k, 0:2, f0:f0 + P],
                                        rhs=x_sb[:, k, 0:2, :],
                                        start=(m == 0), stop=(m == nmm - 1),
                                        perf_mode=DR,
                                    )
                                    m += 1
                            h32 = h32pool.tile([P, 2, NBl], f32, tag="h32")
                            nc.scalar.activation(h32[:], p1[:], Gelu,
                                                 scale=S1)
                            nc.scalar.activation(hp[:, 0:2, 1, :], h32[:],
                                                 Copy)
                            nc.vector.tensor_sub(hp[:, 0:2, 0, :], h32[:],
                                                 hp[:, 0:2, 1, :])
                        else:
                            for d in (0, 1):
                                fo = 2 * i + d
                                w1t, f0 = w1_by_chunk[fo]
                                p1 = psum1.tile([P, NBl], f32, tag="p1")
                                nmm = KO1 // 2 + KO1
                                m = 0
                                # hi-hi: 2 K-chunks per DoubleRow matmul
                                for k in range(KO1 // 2):
                                    nc.tensor.matmul(
                                        p1[:],
                                        lhsT=w1t[:, 2 * k:2 * k + 2, 0,
                                                 f0:f0 + P],
                                        rhs=x_sb[:, 2 * k:2 * k + 2, 1, :],
                                        start=(m == 0), stop=(m == nmm - 1),
                                        perf_mode=DR,
                                    )
                                    m += 1
                                # cross: W1h.T@x_lo + W1l.T@x_hi per K-chunk
                                for k in range(KO1):
                                    nc.tensor.matmul(
                                        p1[:],
                                        lhsT=w1t[:, k, 0:2, f0:f0 + P],
                                        rhs=x_sb[:, k, 0:2, :],
                                        start=(m == 0), stop=(m == nmm - 1),
                                        perf_mode=DR,
                                    )
                                    m += 1
                                h32 = h32pool.tile([P, NBl], f32, tag="h32")
                                nc.scalar.activation(h32[:], p1[:], Gelu,
                                                     bias=b1_sb[:, fo:fo + 1],
                                                     scale=S1)
                                nc.scalar.activation(hp[:, d, 1, :], h32[:],
                                                     Copy)
                                nc.vector.tensor_sub(hp[:, d, 0, :], h32[:],
                                                     hp[:, d, 1, :])
                        h_pairs[i] = hp
                    j = i - LA
                    if j >= 0:
                        # Final block: only the dh=0 half is pipelined here;
                        # dh=1 runs afterwards while dh=0 drains.
                        hp = h_pairs[j] if last else h_pairs.pop(j)
                        w2t, jr = w2_by_pair[j]
                        for ts in range(TS):
                            t0 = ts * P
                            for dh in range(1 if last else DH):
                                d0 = dh * DHW
                                yp = ypsum[ts][dh]
                                # hi-hi: pair of consecutive ko2 chunks
                                nc.tensor.matmul(
                                    yp[:],
                                    lhsT=hp[:, 0:2, 1, t0:t0 + P],
                                    rhs=w2t[:, jr:jr + 2, 0, d0:d0 + DHW],
                                    start=(j == 0), stop=False,
                                    perf_mode=DR,
                                )
                                # cross: h_lo.T@W2h + h_hi.T@W2l per chunk
                                for d in (0, 1):
                                    nc.tensor.matmul(
                                        yp[:],
                                        lhsT=hp[:, d, 0:2, t0:t0 + P],
                                        rhs=w2t[:, jr + d, 0:2, d0:d0 + DHW],
                                        start=False,
                                        stop=(j == JP2 - 1 and d == 1),
                                        perf_mode=DR,
                                    )
                y_sb = ypool.tile([P, TS, D], fy, tag="y")
                yv = None
                if not skip_xydma:
                    yv = y[:, mo0 * D:(mo0 + TS) * D].rearrange(
                        "p (ts d) -> p ts d", ts=TS)
                if last:
                    # dh=0 finished inside the pipeline above: drain it now
                    # (gate multiply + store) while dh=1's matmuls execute.
                    for ts in range(TS):
                        g_ap = gates_sb[:, mo0 + ts: mo0 + ts + 1]
                        nc.vector.tensor_scalar_mul(
                            y_sb[:, ts, 0:DHW], ypsum[ts][0][:], g_ap)
                    if yv is not None:
                        nc.sync.dma_start(yv[:, :, 0:DHW], y_sb[:, :, 0:DHW])
                    for j in range(JP2):
                        hp = h_pairs[j]
                        w2t, jr = w2_by_pair[j]
                        for ts in range(TS):
                            t0 = ts * P
                            yp = ypsum[ts][1]
                            nc.tensor.matmul(
                                yp[:],
                                lhsT=hp[:, 0:2, 1, t0:t0 + P],
                                rhs=w2t[:, jr:jr + 2, 0, DHW:D],
                                start=(j == 0), stop=False,
                                perf_mode=DR,
                            )
                            for d in (0, 1):
                                nc.tensor.matmul(
                                    yp[:],
                                    lhsT=hp[:, d, 0:2, t0:t0 + P],
                                    rhs=w2t[:, jr + d, 0:2, DHW:D],
                                    start=False,
                                    stop=(j == JP2 - 1 and d == 1),
                                    perf_mode=DR,
                                )
                    h_pairs.clear()
                    for ts in range(TS):
                        g_ap = gates_sb[:, mo0 + ts: mo0 + ts + 1]
                        nc.scalar.mul(y_sb[:, ts, DHW:D], ypsum[ts][1][:],
                                      g_ap)
                    if yv is not None:
                        nc.gpsimd.dma_start(yv[:, :, DHW:D],
                                            y_sb[:, :, DHW:D])
                else:
                    for ts in range(TS):
                        g_ap = gates_sb[:, mo0 + ts: mo0 + ts + 1]
                        nc.vector.tensor_scalar_mul(
                            y_sb[:, ts, 0:DHW], ypsum[ts][0][:], g_ap)
                        nc.vector.tensor_scalar_mul(
                            y_sb[:, ts, DHW:D], ypsum[ts][1][:], g_ap)
                    if yv is not None:
                        nc.sync.dma_start(yv, y_sb[:])
                tok0 += NBl
    nc.compile()
    return nc


def kernel(x, Wg, bg, W1, b1, W2, b2):
    from concourse.bass_utils import run_bass_kernel_spmd

    x = np.asarray(x, dtype=np.float32)
    Wg = np.asarray(Wg, dtype=np.float32)
    bg = np.asarray(bg, dtype=np.float32)
    W1 = np.asarray(W1, dtype=np.float32)
    b1 = np.asarray(b1, dtype=np.float32)
    W2 = np.asarray(W2, dtype=np.float32)
    b2 = np.asarray(b2, dtype=np.float32)

    x_flat = x.reshape(-1, D)
    top2, g2 = _route(x_flat, Wg, bg)

    # Dispatch: token lists per expert
    idx_e = []
    gate_e = []
    for e in range(E):
        sel = np.nonzero(top2 == e)
        idx_e.append(sel[0].astype(np.int64))                  # token ids
        gate_e.append(g2[sel[0], sel[1]].astype(np.float32))   # their gates
    counts = [len(i) for i in idx_e]
    C = max(max(counts), 2 * P)
    C = ((C + P - 1) // P) * P

    if C not in _nc_cache:
        # fuse=False: fused mm1 pairs measured slower (coarser ACT overlap)
        _nc_cache[C] = _build_nc(C, fuse=False)
    nc = _nc_cache[C]

    # Global hi/lo fp8 split of the tokens (shared across experts).
    Xh, Xl = _split8(x_flat, SX)      # [N, D] fp8 each
    blocks = _blocks_of(C)
    bl_off = np.cumsum([0] + blocks)[:-1]

    in_maps = []
    for e in range(E):
        n_e = counts[e]
        # x image: [P, ko, hl, C], hl = (lo, hi), then block-major flattened
        # to match the kernel's per-block contiguous DMA slices.
        xe = np.zeros((P, KO1, 2, C), dtype=F8)
        if n_e:
            xe[:, :, 0, :n_e] = (
                Xl[idx_e[e]].T.reshape(KO1, P, n_e).transpose(1, 0, 2))
            xe[:, :, 1, :n_e] = (
                Xh[idx_e[e]].T.reshape(KO1, P, n_e).transpose(1, 0, 2))
        xe = np.concatenate(
            [xe[:, :, :, o:o + nb].reshape(P, -1)
             for o, nb in zip(bl_off, blocks)], axis=1)
        # w1 image: [P, ko, hl, F], hl = (hi, lo), then piece-major
        # flattened to match the kernel's per-piece contiguous DMA slices.
        w1h, w1l = _split8(W1[e], SW)      # [D, F]
        w1e = np.empty((P, KO1, 2, F), dtype=F8)
        w1e[:, :, 0, :] = w1h.reshape(KO1, P, F).transpose(1, 0, 2)
        w1e[:, :, 1, :] = w1l.reshape(KO1, P, F).transpose(1, 0, 2)
        w1e = np.concatenate(
            [w1e[:, :, :, a * P:b * P].reshape(P, -1)
             for a, b in PIECES1], axis=1)
        # w2 image: [P, ko2, hl, D], hl = (hi, lo)
        w2h, w2l = _split8(W2[e], SW)      # [F, D]
        w2e = np.empty((P, KO2, 2, D), dtype=F8)
        w2e[:, :, 0, :] = w2h.reshape(KO2, P, D).transpose(1, 0, 2)
        w2e[:, :, 1, :] = w2l.reshape(KO2, P, D).transpose(1, 0, 2)
        ge = np.zeros((C,), dtype=np.float32)
        ge[:n_e] = gate_e[e] * np.float32(1.0 / SW)
        in_maps.append({
            "xq": xe,
            "w1q": w1e,
            "w2q": w2e.reshape(P, -1),
            "b1": np.ascontiguousarray(b1[e].reshape(KO2, P).T),
            "gates": np.ascontiguousarray(ge.reshape(C // P, P).T),
        })

    res = run_bass_kernel_spmd(nc, in_maps, core_ids=list(range(NCORES)))

    out = np.zeros((N, D), dtype=np.float32)
    for e in range(E):
        n_e = counts[e]
        if n_e:
            ye = np.asarray(res.results[e]["y"]).astype(np.float32)
            ye = ye.reshape(P, C // P, D).transpose(1, 0, 2).reshape(C, D)
            out[idx_e[e]] += ye[:n_e]
    # separable b2 term: sum_k gate_k * b2[e_k]
    if np.any(b2):
        out += g2[:, 0:1] * b2[top2[:, 0]] + g2[:, 1:2] * b2[top2[:, 1]]
    return out.reshape(B, T, D)


# revision 64
# speedup vs baseline: 1.3598x; 1.0035x over previous
"""MoE layer (top-2 routing, E=8 experts) on 8 Trainium2 NeuronCores.

Strategy (expert-parallel, per sharding hint):
 - Host computes the router (softmax over x@Wg+bg, top-2) and dispatches
   each (token, gate) pair to its expert's core: core e gets the tokens
   routed to expert e, zero-padded to a common capacity C.
 - Core e runs a fused MLP for expert e entirely in fp8 (e4m3) matmuls
   using the PE DoubleRow perf mode (2 contraction chunks per matmul at
   0.5 cycles/row).  To keep accuracy well inside the 2e-2 gate each
   GEMM is computed as a 3-term "split fp8" sum with hi/lo e4m3 operands:
       A@B ~= A_hi@B_hi + A_lo@B_hi + A_hi@B_lo       (rel err ~2e-3)
   where X_hi = e4m3(s*X), X_lo = e4m3(s*X - X_hi) at the same scale s.
   Per 128-contraction-chunk this costs 1.5 fp8 products = 0.75 of one
   fp32r matmul, on top of DoubleRow's 2x rate advantage.
 - Token-block pipeline per core: for each 256-token block
       mm1 (9 DoubleRow matmuls / 128-F-chunk) -> PSUM
       ACT: h = gelu(2^-13 * psum + b1)  (fp32), h_hi = e4m3(h)   [Scalar]
       DVE: h_lo = e4m3(h - h_hi)                                 [Vector]
       mm2 (3 DoubleRow matmuls / ko2-pair per (ts,dh)) -> PSUM
       DVE: y = psum * (gate * 2^-8)  -> SBUF -> DMA out
   mm1 of pair i+1 is emitted ahead of mm2 of pair i so the PE never
   stalls on the ACT/DVE h-split latency.
 - Host scatter-adds the per-expert outputs back into [N, D] and adds
   the separable b2 term: sum_k gate_k * b2[e_k].
"""

import numpy as np
import ml_dtypes

B, T, D = 4, 2048, 768
E, F, TOPK = 8, 4 * 768, 2
N = B * T
P = 128
NB = 256            # tokens per on-device block
LA = 2              # mm1 chunk-pair lookahead over the mm2 stream
WARM = 0            # PE warmup dummy matmul count (0 = off)
NCORES = 8
KO1 = D // P        # 6 contraction chunks for x@W1
KO2 = F // P        # 24 contraction chunks for h@W2
JP2 = KO2 // 2      # 12 chunk-pairs for mm2
# Weight DMA piece bounds in 128-chunks (w1: F-chunks, w2: ko2-chunks).
# 6-chunk pieces balance DMA pipelining against per-DMA fixed overheads.
# w2 piece bounds must be even (mm2 chunk-pairs may not straddle pieces).
PIECES1 = [(0, 6), (6, 12), (12, 18), (18, 24)]
PIECES2 = [(0, 6), (6, 12), (12, 18), (18, 24)]
DH = 2
DHW = D // DH       # 384
SX = 32.0           # x quant scale (2^5)
SW = 256.0          # weight quant scale (2^8)
F8 = ml_dtypes.float8_e4m3

_nc_cache = {}


def _route(x_flat, Wg, bg):
    """Replicate reference routing: softmax gates, top-2 (ties -> lower idx)."""
    logits = x_flat.astype(np.float64) @ Wg.astype(np.float64) + bg.astype(np.float64)
    logits -= logits.max(axis=-1, keepdims=True)
    eg = np.exp(logits)
    gates = eg / eg.sum(axis=-1, keepdims=True)          # [N, E] f64
    top2 = np.argsort(-gates, axis=-1, kind="stable")[:, :TOPK]   # [N, 2]
    g2 = np.take_along_axis(gates, top2, axis=-1).astype(np.float32)
    return top2, g2


def _q8(a):
    return np.clip(a, -240.0, 240.0).astype(F8)


def _split8(a, s):
    """hi/lo e4m3 split of a at scale s (both at the same scale)."""
    hi = _q8(a * np.float32(s))
    lo = _q8(a * np.float32(s) - hi.astype(np.float32))
    return hi, lo


def _blocks_of(C):
    assert C % P == 0
    nfull, rem = divmod(C, NB)
    return [NB] * nfull + ([rem] if rem else [])


def _build_nc(C, act="Gelu", skip_wdma=False, skip_xydma=False, fuse=False):
    import concourse.bacc as bacc
    import concourse.mybir as mybir
    import concourse.tile as tile

    f32 = mybir.dt.float32
    f8 = mybir.dt.float8e4
    fy = mybir.dt.bfloat16
    Gelu = getattr(mybir.ActivationFunctionType, act)
    Copy = mybir.ActivationFunctionType.Copy
    DR = mybir.MatmulPerfMode.DoubleRow
    S1 = 1.0 / (SX * SW)

    blocks = _blocks_of(C)

    nc = bacc.Bacc("TRN2", target_bir_lowering=False)

    # DRAM inputs are pre-laid-out on the host as SBUF images.
    xq = nc.dram_tensor("xq", [P, KO1 * 2 * C], f8, kind="ExternalInput")
    w1q = nc.dram_tensor("w1q", [P, KO1 * 2 * F], f8, kind="ExternalInput")
    w2q = nc.dram_tensor("w2q", [P, KO2 * 2 * D], f8, kind="ExternalInput")
    b1 = nc.dram_tensor("b1", [P, KO2], f32, kind="ExternalInput")
    gates = nc.dram_tensor("gates", [P, C // P], f32, kind="ExternalInput")
    y = nc.dram_tensor("y", [P, (C // P) * D], fy, kind="ExternalOutput")

    w2_view = w2q[:, :].rearrange("p (ko hl d) -> p ko hl d", ko=KO2, hl=2)

    with tile.TileContext(nc) as tc:
        with (
            tc.tile_pool(name="wpool", bufs=1) as wpool,
            tc.tile_pool(name="xpool", bufs=2) as xpool,
            tc.tile_pool(name="hpool", bufs=JP2 + 2) as hpool,
            tc.tile_pool(name="h32pool", bufs=4) as h32pool,
            tc.tile_pool(name="ypool", bufs=2) as ypool,
            tc.tile_pool(name="psum1", bufs=4, space="PSUM") as psum1,
            tc.tile_pool(name="psumy", bufs=1, space="PSUM") as psumy,
        ):
            b1_sb = wpool.tile([P, KO2], f32, tag="b1")
            gates_sb = wpool.tile([P, C // P], f32, tag="gates")
            # PE warmup: dummy DoubleRow matmuls on a zeroed tile keep the PE
            # "continuously busy" through the cost model's p-state ramp while
            # the first weight pieces are still in flight.
            if WARM:
                warm = wpool.tile([P, 2, P], f8, tag="warm")
                nc.vector.memset(warm[:], 0)
                wpsum = psum1.tile([P, NB], f32, tag="p1")
                for _ in range(WARM):
                    nc.tensor.matmul(wpsum[:, 0:P], lhsT=warm[:], rhs=warm[:],
                                     start=True, stop=True, perf_mode=DR)
            # Quantized weights, DMA'd in consumption order so the PE can
            # start almost immediately.
            # x block fetches go on the gpsimd (Pool) DGE, y stores on the
            # SP DGE: with both on one engine, block b+1's x fetch issues
            # in-order behind block b's y store and the PE stalls ~1.3us at
            # every block boundary.
            x_tiles = {}

            def fetch_x(tok0, NBl):
                t = xpool.tile([P, KO1, 2, NBl], f8, tag="x")
                off = KO1 * 2 * tok0
                if not skip_xydma:
                    nc.gpsimd.dma_start(
                        t[:],
                        xq[:, off:off + KO1 * 2 * NBl].rearrange(
                            "p (ko hl n) -> p ko hl n", ko=KO1, hl=2))
                return t

            # Block 0's tokens are needed before any weights: fetch first so
            # the transfer isn't queued behind the first weight pieces.
            x_tiles[0] = fetch_x(0, blocks[0])

            # Interleave w1/w2 pieces roughly in consumption order.
            order = []
            i1 = i2 = 0
            while i1 < len(PIECES1) or i2 < len(PIECES2):
                if i1 < len(PIECES1):
                    order.append((1, i1)); i1 += 1
                if i2 < len(PIECES2):
                    order.append((2, i2)); i2 += 1
            w1_off = np.cumsum(
                [0] + [KO1 * 2 * (b - a) * P for a, b in PIECES1])
            w1_by_chunk = {}
            w2_by_pair = {}
            for oi, (which, q) in enumerate(order):
                if oi == 1:
                    # First weight piece gets the head of the SP DGE queue;
                    # the small constants follow (needed slightly later).
                    nc.sync.dma_start(b1_sb[:], b1[:, :])
                    nc.sync.dma_start(gates_sb[:], gates[:, :])
                if which == 1:
                    a, b = PIECES1[q]
                    t1 = wpool.tile([P, KO1, 2, (b - a) * P], f8,
                                    tag=f"w1_{q}", name=f"w1sb_{q}")
                    if not skip_wdma:
                        # w1q is piece-major on the host: piece q's rows are
                        # contiguous, so descriptors stay >= 512B.
                        nc.sync.dma_start(
                            t1[:],
                            w1q[:, w1_off[q]:w1_off[q + 1]].rearrange(
                                "p (ko hl f) -> p ko hl f", ko=KO1, hl=2))
                    for fo in range(a, b):
                        w1_by_chunk[fo] = (t1, (fo - a) * P)
                else:
                    a, b = PIECES2[q]
                    t2 = wpool.tile([P, b - a, 2, D], f8,
                                    tag=f"w2_{q}", name=f"w2sb_{q}")
                    if not skip_wdma:
                        nc.sync.dma_start(t2[:], w2_view[:, a:b])
                    for j in range(a // 2, b // 2):
                        w2_by_pair[j] = (t2, 2 * j - a)

            # Flat cross-block software pipeline over global chunk-pair
            # slots: mm1 slot g belongs to block g // JP2 and the mm2 stream
            # trails by LA slots, so block b+1's first mm1 pairs interleave
            # with block b's last mm2 pairs and the ACT/DVE h-split never
            # goes idle at block boundaries.
            nb = len(blocks)
            boff = [0]
            for nbl in blocks:
                boff.append(boff[-1] + nbl)
            st = {}

            def prologue(b):
                NBl = blocks[b]
                TS = NBl // P
                xt = x_tiles.pop(b) if b in x_tiles else fetch_x(boff[b], NBl)
                yp = [[psumy.tile([P, DHW], f32, tag=f"y_{ts}_{dh}",
                                  name=f"ypsum_{ts}_{dh}")
                       for dh in range(DH)] for ts in range(TS)]
                st[b] = {"x": xt, "yp": yp, "h": {}}

            def mm1_unit(b, i):
                s = st[b]
                NBl = blocks[b]
                x_sb = s["x"]
                hp = hpool.tile([P, 2, 2, NBl], f8, tag="h")
                for d in (0, 1):
                    fo = 2 * i + d
                    w1t, f0 = w1_by_chunk[fo]
                    p1 = psum1.tile([P, NBl], f32, tag="p1")
                    nmm = KO1 // 2 + KO1
                    m = 0
                    # hi-hi terms: 2 K-chunks per DoubleRow matmul
                    for k in range(KO1 // 2):
                        nc.tensor.matmul(
                            p1[:],
                            lhsT=w1t[:, 2 * k:2 * k + 2, 0, f0:f0 + P],
                            rhs=x_sb[:, 2 * k:2 * k + 2, 1, :],
                            start=(m == 0), stop=(m == nmm - 1),
                            perf_mode=DR,
                        )
                        m += 1
                    # cross terms: W1h.T@x_lo + W1l.T@x_hi per K-chunk
                    for k in range(KO1):
                        nc.tensor.matmul(
                            p1[:],
                            lhsT=w1t[:, k, 0:2, f0:f0 + P],
                            rhs=x_sb[:, k, 0:2, :],
                            start=(m == 0), stop=(m == nmm - 1),
                            perf_mode=DR,
                        )
                        m += 1
                    h32 = h32pool.tile([P, NBl], f32, tag="h32")
                    nc.scalar.activation(h32[:], p1[:], Gelu,
                                         bias=b1_sb[:, fo:fo + 1], scale=S1)
                    nc.scalar.activation(hp[:, d, 1, :], h32[:], Copy)
                    nc.vector.tensor_sub(hp[:, d, 0, :], h32[:],
                                         hp[:, d, 1, :])
                s["h"][i] = hp

            def mm2_unit(b, j):
                s = st[b]
                lastb = b == nb - 1
                TS = blocks[b] // P
                # Final block: only the dh=0 half is pipelined here; dh=1
                # runs in the epilogue while dh=0 drains.
                hp = s["h"][j] if lastb else s["h"].pop(j)
                w2t, jr = w2_by_pair[j]
                for ts in range(TS):
                    t0 = ts * P
                    for dh in range(1 if lastb else DH):
                        d0 = dh * DHW
                        yp = s["yp"][ts][dh]
                        # hi-hi: pair of consecutive ko2 chunks
                        nc.tensor.matmul(
                            yp[:],
                            lhsT=hp[:, 0:2, 1, t0:t0 + P],
                            rhs=w2t[:, jr:jr + 2, 0, d0:d0 + DHW],
                            start=(j == 0), stop=False,
                            perf_mode=DR,
                        )
                        # cross: h_lo.T@W2h + h_hi.T@W2l per chunk
                        for d in (0, 1):
                            nc.tensor.matmul(
                                yp[:],
                                lhsT=hp[:, d, 0:2, t0:t0 + P],
                                rhs=w2t[:, jr + d, 0:2, d0:d0 + DHW],
                                start=False,
                                stop=(j == JP2 - 1 and d == 1),
                                perf_mode=DR,
                            )

            def epilogue(b):
                s = st.pop(b)
                NBl = blocks[b]
                TS = NBl // P
                mo0 = boff[b] // P
                lastb = b == nb - 1
                ypsum = s["yp"]
                y_sb = ypool.tile([P, TS, D], fy, tag="y")
                yv = None
                if not skip_xydma:
                    yv = y[:, mo0 * D:(mo0 + TS) * D].rearrange(
                        "p (ts d) -> p ts d", ts=TS)
                if lastb:
                    # dh=0 finished in the pipeline: drain it now (gate
                    # multiply + store) while dh=1's matmuls execute.
                    for ts in range(TS):
                        g_ap = gates_sb[:, mo0 + ts: mo0 + ts + 1]
                        nc.vector.tensor_scalar_mul(
                            y_sb[:, ts, 0:DHW], ypsum[ts][0][:], g_ap)
                    if yv is not None:
                        nc.sync.dma_start(yv[:, :, 0:DHW], y_sb[:, :, 0:DHW])
                    for j in range(JP2):
                        hp = s["h"][j]
                        w2t, jr = w2_by_pair[j]
                        for ts in range(TS):
                            t0 = ts * P
                            yp = ypsum[ts][1]
                            nc.tensor.matmul(
                                yp[:],
                                lhsT=hp[:, 0:2, 1, t0:t0 + P],
                                rhs=w2t[:, jr:jr + 2, 0, DHW:D],
                                start=(j == 0), stop=False,
                                perf_mode=DR,
                            )
                            for d in (0, 1):
                                nc.tensor.matmul(
                                    yp[:],
                                    lhsT=hp[:, d, 0:2, t0:t0 + P],
                                    rhs=w2t[:, jr + d, 0:2, DHW:D],
                                    start=False,
                                    stop=(j == JP2 - 1 and d == 1),
                                    perf_mode=DR,
                                )
                    s["h"].clear()
                    for ts in range(TS):
                        g_ap = gates_sb[:, mo0 + ts: mo0 + ts + 1]
                        nc.scalar.mul(y_sb[:, ts, DHW:D], ypsum[ts][1][:],
                                      g_ap)
                    if yv is not None:
                        nc.gpsimd.dma_start(yv[:, :, DHW:D],
                                            y_sb[:, :, DHW:D])
                else:
                    for ts in range(TS):
                        g_ap = gates_sb[:, mo0 + ts: mo0 + ts + 1]
                        nc.vector.tensor_scalar_mul(
                            y_sb[:, ts, 0:DHW], ypsum[ts][0][:], g_ap)
                        nc.vector.tensor_scalar_mul(
                            y_sb[:, ts, DHW:D], ypsum[ts][1][:], g_ap)
                    if yv is not None:
                        nc.sync.dma_start(yv, y_sb[:])

            for g in range(nb * JP2 + LA):
                if g < nb * JP2:
                    b, i = divmod(g, JP2)
                    if i == 0:
                        prologue(b)
                    mm1_unit(b, i)
                sl = g - LA
                if sl >= 0:
                    b2, j2 = divmod(sl, JP2)
                    mm2_unit(b2, j2)
                    if j2 == JP2 - 1:
                        epilogue(b2)
    nc.compile()
    return nc


def kernel(x, Wg, bg, W1, b1, W2, b2):
    from concourse.bass_utils import run_bass_kernel_spmd

    x = np.asarray(x, dtype=np.float32)
    Wg = np.asarray(Wg, dtype=np.float32)
    bg = np.asarray(bg, dtype=np.float32)
    W1 = np.asarray(W1, dtype=np.float32)
    b1 = np.asarray(b1, dtype=np.float32)
    W2 = np.asarray(W2, dtype=np.float32)
    b2 = np.asarray(b2, dtype=np.float32)

    x_flat = x.reshape(-1, D)
    top2, g2 = _route(x_flat, Wg, bg)

    # Dispatch: token lists per expert
    idx_e = []
    gate_e = []
    for e in range(E):
        sel = np.nonzero(top2 == e)
        idx_e.append(sel[0].astype(np.int64))                  # token ids
        gate_e.append(g2[sel[0], sel[1]].astype(np.float32))   # their gates
    counts = [len(i) for i in idx_e]
    C = max(max(counts), 2 * P)
    C = ((C + P - 1) // P) * P

    if C not in _nc_cache:
        # fuse=False: fused mm1 pairs measured slower (coarser ACT overlap)
        _nc_cache[C] = _build_nc(C, fuse=False)
    nc = _nc_cache[C]

    # Global hi/lo fp8 split of the tokens (shared across experts).
    Xh, Xl = _split8(x_flat, SX)      # [N, D] fp8 each
    blocks = _blocks_of(C)
    bl_off = np.cumsum([0] + blocks)[:-1]

    in_maps = []
    for e in range(E):
        n_e = counts[e]
        # x image: [P, ko, hl, C], hl = (lo, hi), then block-major flattened
        # to match the kernel's per-block contiguous DMA slices.
        xe = np.zeros((P, KO1, 2, C), dtype=F8)
        if n_e:
            xe[:, :, 0, :n_e] = (
                Xl[idx_e[e]].T.reshape(KO1, P, n_e).transpose(1, 0, 2))
            xe[:, :, 1, :n_e] = (
                Xh[idx_e[e]].T.reshape(KO1, P, n_e).transpose(1, 0, 2))
        xe = np.concatenate(
            [xe[:, :, :, o:o + nb].reshape(P, -1)
             for o, nb in zip(bl_off, blocks)], axis=1)
        # w1 image: [P, ko, hl, F], hl = (hi, lo), then piece-major
        # flattened to match the kernel's per-piece contiguous DMA slices.
        w1h, w1l = _split8(W1[e], SW)      # [D, F]
        w1e = np.empty((P, KO1, 2, F), dtype=F8)
        w1e[:, :, 0, :] = w1h.reshape(KO1, P, F).transpose(1, 0, 2)
        w1e[:, :, 1, :] = w1l.reshape(KO1, P, F).transpose(1, 0, 2)
        w1e = np.concatenate(
            [w1e[:, :, :, a * P:b * P].reshape(P, -1)
             for a, b in PIECES1], axis=1)
        # w2 image: [P, ko2, hl, D], hl = (hi, lo)
        w2h, w2l = _split8(W2[e], SW)      # [F, D]
        w2e = np.empty((P, KO2, 2, D), dtype=F8)
        w2e[:, :, 0, :] = w2h.reshape(KO2, P, D).transpose(1, 0, 2)
        w2e[:, :, 1, :] = w2l.reshape(KO2, P, D).transpose(1, 0, 2)
        ge = np.zeros((C,), dtype=np.float32)
        ge[:n_e] = gate_e[e] * np.float32(1.0 / SW)
        in_maps.append({
            "xq": xe,
            "w1q": w1e,
            "w2q": w2e.reshape(P, -1),
            "b1": np.ascontiguousarray(b1[e].reshape(KO2, P).T),
            "gates": np.ascontiguousarray(ge.reshape(C // P, P).T),
        })

    res = run_bass_kernel_spmd(nc, in_maps, core_ids=list(range(NCORES)))

    out = np.zeros((N, D), dtype=np.float32)
    for e in range(E):
        n_e = counts[e]
        if n_e:
            ye = np.asarray(res.results[e]["y"]).astype(np.float32)
            ye = ye.reshape(P, C // P, D).transpose(1, 0, 2).reshape(C, D)
            out[idx_e[e]] += ye[:n_e]
    # separable b2 term: sum_k gate_k * b2[e_k]
    if np.any(b2):
        out += g2[:, 0:1] * b2[top2[:, 0]] + g2[:, 1:2] * b2[top2[:, 1]]
    return out.reshape(B, T, D)


# revision 66
# speedup vs baseline: 1.3704x; 1.0078x over previous
"""MoE layer (top-2 routing, E=8 experts) on 8 Trainium2 NeuronCores.

Strategy (expert-parallel, per sharding hint):
 - Host computes the router (softmax over x@Wg+bg, top-2) and dispatches
   each (token, gate) pair to its expert's core: core e gets the tokens
   routed to expert e, zero-padded to a common capacity C.
 - Core e runs a fused MLP for expert e entirely in fp8 (e4m3) matmuls
   using the PE DoubleRow perf mode (2 contraction chunks per matmul at
   0.5 cycles/row).  To keep accuracy well inside the 2e-2 gate each
   GEMM is computed as a 3-term "split fp8" sum with hi/lo e4m3 operands:
       A@B ~= A_hi@B_hi + A_lo@B_hi + A_hi@B_lo       (rel err ~2e-3)
   where X_hi = e4m3(s*X), X_lo = e4m3(s*X - X_hi) at the same scale s.
   Per 128-contraction-chunk this costs 1.5 fp8 products = 0.75 of one
   fp32r matmul, on top of DoubleRow's 2x rate advantage.
 - Token-block pipeline per core: for each 256-token block
       mm1 (9 DoubleRow matmuls / 128-F-chunk) -> PSUM
       ACT: h = gelu(2^-13 * psum + b1)  (fp32), h_hi = e4m3(h)   [Scalar]
       DVE: h_lo = e4m3(h - h_hi)                                 [Vector]
       mm2 (3 DoubleRow matmuls / ko2-pair per (ts,dh)) -> PSUM
       DVE: y = psum * (gate * 2^-8)  -> SBUF -> DMA out
   mm1 of pair i+1 is emitted ahead of mm2 of pair i so the PE never
   stalls on the ACT/DVE h-split latency.
 - Host scatter-adds the per-expert outputs back into [N, D] and adds
   the separable b2 term: sum_k gate_k * b2[e_k].
"""

import numpy as np
import ml_dtypes

B, T, D = 4, 2048, 768
E, F, TOPK = 8, 4 * 768, 2
N = B * T
P = 128
NB = 256            # tokens per on-device block
LA = 2              # mm1 chunk-pair lookahead over the mm2 stream
WARM = 0            # PE warmup dummy matmul count (0 = off)
NCORES = 8
KO1 = D // P        # 6 contraction chunks for x@W1
KO2 = F // P        # 24 contraction chunks for h@W2
JP2 = KO2 // 2      # 12 chunk-pairs for mm2
# Weight DMA piece bounds in 128-chunks (w1: F-chunks, w2: ko2-chunks).
# 6-chunk pieces balance DMA pipelining against per-DMA fixed overheads.
# w2 piece bounds must be even (mm2 chunk-pairs may not straddle pieces).
PIECES1 = [(0, 6), (6, 12), (12, 18), (18, 24)]
PIECES2 = [(0, 6), (6, 12), (12, 18), (18, 24)]
DH = 2
DHW = D // DH       # 384
SX = 32.0           # x quant scale (2^5)
SW = 256.0          # weight quant scale (2^8)
F8 = ml_dtypes.float8_e4m3

_nc_cache = {}


def _route(x_flat, Wg, bg):
    """Replicate reference routing: softmax gates, top-2 (ties -> lower idx)."""
    logits = x_flat.astype(np.float64) @ Wg.astype(np.float64) + bg.astype(np.float64)
    logits -= logits.max(axis=-1, keepdims=True)
    eg = np.exp(logits)
    gates = eg / eg.sum(axis=-1, keepdims=True)          # [N, E] f64
    top2 = np.argsort(-gates, axis=-1, kind="stable")[:, :TOPK]   # [N, 2]
    g2 = np.take_along_axis(gates, top2, axis=-1).astype(np.float32)
    return top2, g2


def _q8(a):
    return np.clip(a, -240.0, 240.0).astype(F8)


def _split8(a, s):
    """hi/lo e4m3 split of a at scale s (both at the same scale)."""
    hi = _q8(a * np.float32(s))
    lo = _q8(a * np.float32(s) - hi.astype(np.float32))
    return hi, lo


def _blocks_of(C):
    assert C % P == 0
    nfull, rem = divmod(C, NB)
    return [NB] * nfull + ([rem] if rem else [])


def _build_nc(C, act="Gelu", skip_wdma=False, skip_xydma=False, fuse=False):
    import concourse.bacc as bacc
    import concourse.mybir as mybir
    import concourse.tile as tile

    f32 = mybir.dt.float32
    f8 = mybir.dt.float8e4
    fy = mybir.dt.bfloat16
    Gelu = getattr(mybir.ActivationFunctionType, act)
    Copy = mybir.ActivationFunctionType.Copy
    DR = mybir.MatmulPerfMode.DoubleRow
    S1 = 1.0 / (SX * SW)

    blocks = _blocks_of(C)

    nc = bacc.Bacc("TRN2", target_bir_lowering=False)

    # DRAM inputs are pre-laid-out on the host as SBUF images.
    xq = nc.dram_tensor("xq", [P, KO1 * 2 * C], f8, kind="ExternalInput")
    w1q = nc.dram_tensor("w1q", [P, KO1 * 2 * F], f8, kind="ExternalInput")
    w2q = nc.dram_tensor("w2q", [P, KO2 * 2 * D], f8, kind="ExternalInput")
    b1 = nc.dram_tensor("b1", [P, KO2], f32, kind="ExternalInput")
    gates = nc.dram_tensor("gates", [P, C // P], f32, kind="ExternalInput")
    y = nc.dram_tensor("y", [P, (C // P) * D], fy, kind="ExternalOutput")

    w2_view = w2q[:, :].rearrange("p (ko hl d) -> p ko hl d", ko=KO2, hl=2)

    with tile.TileContext(nc) as tc:
        with (
            tc.tile_pool(name="wpool", bufs=1) as wpool,
            tc.tile_pool(name="xpool", bufs=2) as xpool,
            tc.tile_pool(name="hpool", bufs=JP2 + 2) as hpool,
            tc.tile_pool(name="h32pool", bufs=4) as h32pool,
            tc.tile_pool(name="ypool", bufs=2) as ypool,
            tc.tile_pool(name="psum1", bufs=4, space="PSUM") as psum1,
            tc.tile_pool(name="psumy", bufs=1, space="PSUM") as psumy,
        ):
            b1_sb = wpool.tile([P, KO2], f32, tag="b1")
            gates_sb = wpool.tile([P, C // P], f32, tag="gates")
            # PE warmup: dummy DoubleRow matmuls on a zeroed tile keep the PE
            # "continuously busy" through the cost model's p-state ramp while
            # the first weight pieces are still in flight.
            if WARM:
                warm = wpool.tile([P, 2, P], f8, tag="warm")
                nc.vector.memset(warm[:], 0)
                wpsum = psum1.tile([P, NB], f32, tag="p1")
                for _ in range(WARM):
                    nc.tensor.matmul(wpsum[:, 0:P], lhsT=warm[:], rhs=warm[:],
                                     start=True, stop=True, perf_mode=DR)
            # Quantized weights, DMA'd in consumption order so the PE can
            # start almost immediately.
            # x block fetches go on the gpsimd (Pool) DGE, y stores on the
            # SP DGE: with both on one engine, block b+1's x fetch issues
            # in-order behind block b's y store and the PE stalls ~1.3us at
            # every block boundary.
            x_tiles = {}

            def fetch_x(tok0, NBl):
                t = xpool.tile([P, KO1, 2, NBl], f8, tag="x")
                off = KO1 * 2 * tok0
                if not skip_xydma:
                    nc.gpsimd.dma_start(
                        t[:],
                        xq[:, off:off + KO1 * 2 * NBl].rearrange(
                            "p (ko hl n) -> p ko hl n", ko=KO1, hl=2))
                return t

            # Block 0's tokens are needed before any weights: fetch first so
            # the transfer isn't queued behind the first weight pieces.
            x_tiles[0] = fetch_x(0, blocks[0])

            # Interleave w1/w2 pieces roughly in consumption order.
            order = []
            i1 = i2 = 0
            while i1 < len(PIECES1) or i2 < len(PIECES2):
                if i1 < len(PIECES1):
                    order.append((1, i1)); i1 += 1
                if i2 < len(PIECES2):
                    order.append((2, i2)); i2 += 1
            w1_off = np.cumsum(
                [0] + [KO1 * 2 * (b - a) * P for a, b in PIECES1])
            w1_by_chunk = {}
            w2_by_pair = {}
            for oi, (which, q) in enumerate(order):
                if oi == 1:
                    # First weight piece gets the head of the SP DGE queue;
                    # the small constants follow (needed slightly later).
                    nc.sync.dma_start(b1_sb[:], b1[:, :])
                    nc.sync.dma_start(gates_sb[:], gates[:, :])
                if which == 1:
                    a, b = PIECES1[q]
                    t1 = wpool.tile([P, KO1, 2, (b - a) * P], f8,
                                    tag=f"w1_{q}", name=f"w1sb_{q}")
                    if not skip_wdma:
                        # w1q is piece-major on the host: piece q's rows are
                        # contiguous, so descriptors stay >= 512B.
                        nc.sync.dma_start(
                            t1[:],
                            w1q[:, w1_off[q]:w1_off[q + 1]].rearrange(
                                "p (ko hl f) -> p ko hl f", ko=KO1, hl=2))
                    for fo in range(a, b):
                        w1_by_chunk[fo] = (t1, (fo - a) * P)
                else:
                    a, b = PIECES2[q]
                    t2 = wpool.tile([P, b - a, 2, D], f8,
                                    tag=f"w2_{q}", name=f"w2sb_{q}")
                    if not skip_wdma:
                        nc.sync.dma_start(t2[:], w2_view[:, a:b])
                    for j in range(a // 2, b // 2):
                        w2_by_pair[j] = (t2, 2 * j - a)

            # Flat cross-block software pipeline over global chunk-pair
            # slots: mm1 slot g belongs to block g // JP2 and the mm2 stream
            # trails by LA slots, so block b+1's first mm1 pairs interleave
            # with block b's last mm2 pairs and the ACT/DVE h-split never
            # goes idle at block boundaries.
            nb = len(blocks)
            boff = [0]
            for nbl in blocks:
                boff.append(boff[-1] + nbl)
            st = {}

            def prologue(b):
                NBl = blocks[b]
                TS = NBl // P
                xt = x_tiles.pop(b) if b in x_tiles else fetch_x(boff[b], NBl)
                yp = [[psumy.tile([P, DHW], f32, tag=f"y_{ts}_{dh}",
                                  name=f"ypsum_{ts}_{dh}")
                       for dh in range(DH)] for ts in range(TS)]
                st[b] = {"x": xt, "yp": yp, "h": {}}

            def mm1_unit(b, i):
                s = st[b]
                NBl = blocks[b]
                lastb = b == nb - 1
                x_sb = s["x"]
                hp = hpool.tile([P, 2, 2, NBl], f8, tag="h")
                for d in (0, 1):
                    fo = 2 * i + d
                    w1t, f0 = w1_by_chunk[fo]
                    p1 = psum1.tile([P, NBl], f32, tag="p1")
                    nmm = KO1 // 2 + KO1
                    m = 0
                    # hi-hi terms: 2 K-chunks per DoubleRow matmul
                    for k in range(KO1 // 2):
                        nc.tensor.matmul(
                            p1[:],
                            lhsT=w1t[:, 2 * k:2 * k + 2, 0, f0:f0 + P],
                            rhs=x_sb[:, 2 * k:2 * k + 2, 1, :],
                            start=(m == 0), stop=(m == nmm - 1),
                            perf_mode=DR,
                        )
                        m += 1
                    # cross terms: W1h.T@x_lo + W1l.T@x_hi per K-chunk
                    for k in range(KO1):
                        nc.tensor.matmul(
                            p1[:],
                            lhsT=w1t[:, k, 0:2, f0:f0 + P],
                            rhs=x_sb[:, k, 0:2, :],
                            start=(m == 0), stop=(m == nmm - 1),
                            perf_mode=DR,
                        )
                        m += 1
                    h32 = h32pool.tile([P, NBl], f32, tag="h32")
                    nc.scalar.activation(h32[:], p1[:], Gelu,
                                         bias=b1_sb[:, fo:fo + 1], scale=S1)
                    if lastb and d == 1:
                        # In the (small) final block the ACT engine is the
                        # h-split critical path: offload half the h_hi casts
                        # to the otherwise-idle gpsimd engine.
                        nc.gpsimd.tensor_copy(hp[:, d, 1, :], h32[:])
                    else:
                        nc.scalar.activation(hp[:, d, 1, :], h32[:], Copy)
                    nc.vector.tensor_sub(hp[:, d, 0, :], h32[:],
                                         hp[:, d, 1, :])
                s["h"][i] = hp

            def mm2_unit(b, j):
                s = st[b]
                lastb = b == nb - 1
                TS = blocks[b] // P
                # Final block: only the dh=0 half is pipelined here; dh=1
                # runs in the epilogue while dh=0 drains.
                hp = s["h"][j] if lastb else s["h"].pop(j)
                w2t, jr = w2_by_pair[j]
                for ts in range(TS):
                    t0 = ts * P
                    for dh in range(1 if lastb else DH):
                        d0 = dh * DHW
                        yp = s["yp"][ts][dh]
                        # hi-hi: pair of consecutive ko2 chunks
                        nc.tensor.matmul(
                            yp[:],
                            lhsT=hp[:, 0:2, 1, t0:t0 + P],
                            rhs=w2t[:, jr:jr + 2, 0, d0:d0 + DHW],
                            start=(j == 0), stop=False,
                            perf_mode=DR,
                        )
                        # cross: h_lo.T@W2h + h_hi.T@W2l per chunk
                        for d in (0, 1):
                            nc.tensor.matmul(
                                yp[:],
                                lhsT=hp[:, d, 0:2, t0:t0 + P],
                                rhs=w2t[:, jr + d, 0:2, d0:d0 + DHW],
                                start=False,
                                stop=(j == JP2 - 1 and d == 1),
                                perf_mode=DR,
                            )

            def epilogue(b):
                s = st.pop(b)
                NBl = blocks[b]
                TS = NBl // P
                mo0 = boff[b] // P
                lastb = b == nb - 1
                ypsum = s["yp"]
                y_sb = ypool.tile([P, TS, D], fy, tag="y")
                yv = None
                if not skip_xydma:
                    yv = y[:, mo0 * D:(mo0 + TS) * D].rearrange(
                        "p (ts d) -> p ts d", ts=TS)
                if lastb:
                    # dh=0 finished in the pipeline: drain it now (gate
                    # multiply + store) while dh=1's matmuls execute.
                    for ts in range(TS):
                        g_ap = gates_sb[:, mo0 + ts: mo0 + ts + 1]
                        nc.vector.tensor_scalar_mul(
                            y_sb[:, ts, 0:DHW], ypsum[ts][0][:], g_ap)
                    if yv is not None:
                        nc.sync.dma_start(yv[:, :, 0:DHW], y_sb[:, :, 0:DHW])
                    for j in range(JP2):
                        hp = s["h"][j]
                        w2t, jr = w2_by_pair[j]
                        for ts in range(TS):
                            t0 = ts * P
                            yp = ypsum[ts][1]
                            nc.tensor.matmul(
                                yp[:],
                                lhsT=hp[:, 0:2, 1, t0:t0 + P],
                                rhs=w2t[:, jr:jr + 2, 0, DHW:D],
                                start=(j == 0), stop=False,
                                perf_mode=DR,
                            )
                            for d in (0, 1):
                                nc.tensor.matmul(
                                    yp[:],
                                    lhsT=hp[:, d, 0:2, t0:t0 + P],
                                    rhs=w2t[:, jr + d, 0:2, DHW:D],
                                    start=False,
                                    stop=(j == JP2 - 1 and d == 1),
                                    perf_mode=DR,
                                )
                    s["h"].clear()
                    for ts in range(TS):
                        g_ap = gates_sb[:, mo0 + ts: mo0 + ts + 1]
                        nc.scalar.mul(y_sb[:, ts, DHW:D], ypsum[ts][1][:],
                                      g_ap)
                    if yv is not None:
                        nc.gpsimd.dma_start(yv[:, :, DHW:D],
                                            y_sb[:, :, DHW:D])
                else:
                    for ts in range(TS):
                        g_ap = gates_sb[:, mo0 + ts: mo0 + ts + 1]
                        nc.vector.tensor_scalar_mul(
                            y_sb[:, ts, 0:DHW], ypsum[ts][0][:], g_ap)
                        nc.vector.tensor_scalar_mul(
                            y_sb[:, ts, DHW:D], ypsum[ts][1][:], g_ap)
                    if yv is not None:
                        nc.sync.dma_start(yv, y_sb[:])

            for g in range(nb * JP2 + LA):
                if g < nb * JP2:
                    b, i = divmod(g, JP2)
                    if i == 0:
                        prologue(b)
                    mm1_unit(b, i)
                sl = g - LA
                if sl >= 0:
                    b2, j2 = divmod(sl, JP2)
                    mm2_unit(b2, j2)
                    if j2 == JP2 - 1:
                        epilogue(b2)
    nc.compile()
    return nc


def kernel(x, Wg, bg, W1, b1, W2, b2):
    from concourse.bass_utils import run_bass_kernel_spmd

    x = np.asarray(x, dtype=np.float32)
    Wg = np.asarray(Wg, dtype=np.float32)
    bg = np.asarray(bg, dtype=np.float32)
    W1 = np.asarray(W1, dtype=np.float32)
    b1 = np.asarray(b1, dtype=np.float32)
    W2 = np.asarray(W2, dtype=np.float32)
    b2 = np.asarray(b2, dtype=np.float32)

    x_flat = x.reshape(-1, D)
    top2, g2 = _route(x_flat, Wg, bg)

    # Dispatch: token lists per expert
    idx_e = []
    gate_e = []
    for e in range(E):
        sel = np.nonzero(top2 == e)
        idx_e.append(sel[0].astype(np.int64))                  # token ids
        gate_e.append(g2[sel[0], sel[1]].astype(np.float32))   # their gates
    counts = [len(i) for i in idx_e]
    C = max(max(counts), 2 * P)
    C = ((C + P - 1) // P) * P

    if C not in _nc_cache:
        # fuse=False: fused mm1 pairs measured slower (coarser ACT overlap)
        _nc_cache[C] = _build_nc(C, fuse=False)
    nc = _nc_cache[C]

    # Global hi/lo fp8 split of the tokens (shared across experts).
    Xh, Xl = _split8(x_flat, SX)      # [N, D] fp8 each
    blocks = _blocks_of(C)
    bl_off = np.cumsum([0] + blocks)[:-1]

    in_maps = []
    for e in range(E):
        n_e = counts[e]
        # x image: [P, ko, hl, C], hl = (lo, hi), then block-major flattened
        # to match the kernel's per-block contiguous DMA slices.
        xe = np.zeros((P, KO1, 2, C), dtype=F8)
        if n_e:
            xe[:, :, 0, :n_e] = (
                Xl[idx_e[e]].T.reshape(KO1, P, n_e).transpose(1, 0, 2))
            xe[:, :, 1, :n_e] = (
                Xh[idx_e[e]].T.reshape(KO1, P, n_e).transpose(1, 0, 2))
        xe = np.concatenate(
            [xe[:, :, :, o:o + nb].reshape(P, -1)
             for o, nb in zip(bl_off, blocks)], axis=1)
        # w1 image: [P, ko, hl, F], hl = (hi, lo), then piece-major
        # flattened to match the kernel's per-piece contiguous DMA slices.
        w1h, w1l = _split8(W1[e], SW)      # [D, F]
        w1e = np.empty((P, KO1, 2, F), dtype=F8)
        w1e[:, :, 0, :] = w1h.reshape(KO1, P, F).transpose(1, 0, 2)
        w1e[:, :, 1, :] = w1l.reshape(KO1, P, F).transpose(1, 0, 2)
        w1e = np.concatenate(
            [w1e[:, :, :, a * P:b * P].reshape(P, -1)
             for a, b in PIECES1], axis=1)
        # w2 image: [P, ko2, hl, D], hl = (hi, lo)
        w2h, w2l = _split8(W2[e], SW)      # [F, D]
        w2e = np.empty((P, KO2, 2, D), dtype=F8)
        w2e[:, :, 0, :] = w2h.reshape(KO2, P, D).transpose(1, 0, 2)
        w2e[:, :, 1, :] = w2l.reshape(KO2, P, D).transpose(1, 0, 2)
        ge = np.zeros((C,), dtype=np.float32)
        ge[:n_e] = gate_e[e] * np.float32(1.0 / SW)
        in_maps.append({
            "xq": xe,
            "w1q": w1e,
            "w2q": w2e.reshape(P, -1),
            "b1": np.ascontiguousarray(b1[e].reshape(KO2, P).T),
            "gates": np.ascontiguousarray(ge.reshape(C // P, P).T),
        })

    res = run_bass_kernel_spmd(nc, in_maps, core_ids=list(range(NCORES)))

    out = np.zeros((N, D), dtype=np.float32)
    for e in range(E):
        n_e = counts[e]
        if n_e:
            ye = np.asarray(res.results[e]["y"]).astype(np.float32)
            ye = ye.reshape(P, C // P, D).transpose(1, 0, 2).reshape(C, D)
            out[idx_e[e]] += ye[:n_e]
    # separable b2 term: sum_k gate_k * b2[e_k]
    if np.any(b2):
        out += g2[:, 0:1] * b2[top2[:, 0]] + g2[:, 1:2] * b2[top2[:, 1]]
    return out.reshape(B, T, D)
